# revision 24
# baseline (speedup 1.0000x reference)
"""CAAM kernel for Trainium2: bf16 single-pass design.

Per-core: one batch element. x arrives host-permuted to bin-blocked
layout [C, 8192] bf16 (free index = bin*1024 + ph*32 + pw). Layouts:
  x resident:  4 SBUF tiles [128, 8192] bf16
  y resident:  4 SBUF tiles [128, 8192] bf16 (attention output, pre-BN)
  camE [19, 8192] bf16: cam, then exp(cam) in place (shares slot with
  phase-C scratch and one phase-F stage buffer)
  ETall [128, 64*19] bf16: exp(cam) transposed per 128-pixel chunk
  stack [(k*8+n), c] bf16 (GCN input), chunk0 rows 0:128, chunk1 0:24
  keyT [128i, 2*152] + val [19, 256] bf16
  per bin: qT [128, 2*1024] bf16, Ehat [19, 1024] bf16 (normalized aff),
  attnT 2x [128, 1024] bf16, y psum -> y resident + sum/sumsq riders.
  BN stats allreduced; phase F: scale/shift + PReLU + residual from
  resident y/x, staged per (bi, cc) for contiguous output DMA.
"""

import numpy as np
import ml_dtypes
import concourse.bass as bass
import concourse.mybir as mybir

F32 = mybir.dt.float32
BF16 = mybir.dt.bfloat16
AX = mybir.AxisListType
OP = mybir.AluOpType
ACT = mybir.ActivationFunctionType

B, C, H, W = 8, 512, 64, 128
K, BH, BW = 19, 2, 4
NB = BH * BW          # 8
CI = C // 2           # 256
HWp = H * W           # 8192
RH, RW = H // BH, W // BW   # 32, 32
P = RH * RW           # 1024
CC = C // 128         # 4
IC = CI // 128        # 2
KN = K * NB           # 152
NPC = HWp // 128      # 64 pixel chunks
EPS = 1e-5

# -------- wpackB column map (bf16 consts) --------
B_IDN = 0        # 128 cols            identity
B_WCAM = 128     # 76 = 4 x 19         conv_cam lhsT chunks
B_W1NK0 = 204    # 152, rows 0:128     gcn conv1 lhsT chunk0
B_W1NK1 = 356    # 152, rows 0:24      chunk1
B_FNK0 = 508     # 19, rows 0:128      fuse lhsT chunk0
B_FNK1 = 527     # 19, rows 0:24       chunk1
B_ONE191 = 546   # 1 col, rows 0:19    ones (sum-over-k lhsT)
B_ONE119 = 547   # 19 cols, row 0      ones (broadcast lhsT)
B_VB = 566       # 256 cols, row 0     v_b
NBW = 822

# -------- wpackF column map (fp32 consts, [128, 22]) --------
F_GANK = 0       # 2: gcn_a-1 per stack row (chunk0, chunk1)
F_CAMB = 2       # 1, rows 0:19
F_FB = 3         # 1, rows 0:19  fuse_b
F_RAM1 = 4       # 1, rows 0:19  relu_a - 1
F_KB = 5         # 2             k_b chunks
F_QB = 7         # 2             q_b chunks
F_GAMMA = 9      # 4
F_BETA = 13      # 4
F_OAM1 = 17      # 4  out_a - 1
F_EPS = 21       # 1
NFW = 22

# -------- dsA ([128, 72]): phase A stats (rows 0:19) --------
A_CS = 0         # 16: cam sums per 512-chunk
A_ES = 16        # 16: exp sums per 512-chunk
A_CSB = 32       # 8: cam sums per bin
A_ESB = 40       # 8: exp sums per bin
A_CLS = 48       # 8: sigmoid(mean cam)
A_REC = 56       # 8: 1/esum
A_SCL = 64       # 8: cls * rec
NA = 72

# -------- dsD ([128, 704]) --------
D_YS = 0         # 64: y sums per (cc, bin, nh)
D_SQ = 64        # 64: y^2 sums per (cc, bin, nh)
D_ST = 128       # 8: packed allreduce input (sum, sumsq per cc)
D_SBN = 136      # 8: allreduce output
D_MOM = 144      # 8
D_VAR = 152      # 4
D_MUSQ = 156     # 4
D_SD = 160       # 4
D_RSTD = 164     # 4
D_SCOL = 168     # 4
D_BCOL = 172     # 4
D_NSC = 176      # 4
D_RROW = 192     # 1024, row 0
ND = 1216

# -------- scr column map (phase-C scratch, bf16, shares camE slot) ----
S_VA = 0         # 512   prelu'd t, chunk0
S_VB = 512       # 512, rows 0:24  chunk1
S_UG = 1024      # 512   u scratch (fp32 would be better but bf16 ok)
S_MG = 1536      # 512
S_TT = 2048      # 608 = 4 x 152   t transposed
S_L2A = 2656     # 512
S_L2B = 3168     # 512, rows 0:24
S_GL = 3680      # 512, rows 0:19  glob (prelu'd)
S_UG2 = 4192     # 512
S_MG2 = 4704     # 512
S_GT = 5216      # 76 = 4 x 19     glob transposed
S_L2T = 5292     # 608             local2 transposed
NS = 5900

# attw pack: keyT 0:304, val 304:560
AT_KEYT = 0
AT_VAL = 304
NAT = 560


def host_prep(wts: dict) -> dict:
    w1 = np.asarray(wts["gcn_w1"], np.float32)
    ga = np.asarray(wts["gcn_a"], np.float32)
    fw = np.asarray(wts["fuse_w"], np.float32).reshape(-1)
    fb = float(np.asarray(wts["fuse_b"], np.float32).reshape(-1)[0])
    ra = float(np.asarray(wts["relu_a"], np.float32).reshape(-1)[0])

    wB = np.zeros((128, NBW), np.float32)
    wB[:, B_IDN:B_IDN + 128] = np.eye(128, dtype=np.float32)
    wcamT = np.asarray(wts["conv_cam_w"], np.float32).T    # [512, 19]
    for cc in range(CC):
        wB[:, B_WCAM + cc*K:B_WCAM + (cc+1)*K] = wcamT[cc*128:(cc+1)*128]
    # conv1 lhsT: W[(m*19+kp), (n*19+k)] = w1[n, m] * (kp == k)
    W1NK = np.zeros((KN, KN), np.float32)
    FNK = np.zeros((KN, K), np.float32)
    ga_nk = np.zeros(KN, np.float32)
    for n in range(NB):
        for k in range(K):
            for m in range(NB):
                W1NK[m*K + k, n*K + k] = w1[n, m]
            FNK[n*K + k, k] = fw[n]
            ga_nk[n*K + k] = ga[n] - 1.0
    wB[:, B_W1NK0:B_W1NK0 + KN] = W1NK[0:128]
    wB[0:24, B_W1NK1:B_W1NK1 + KN] = W1NK[128:KN]
    wB[:, B_FNK0:B_FNK0 + K] = FNK[0:128]
    wB[0:24, B_FNK1:B_FNK1 + K] = FNK[128:KN]
    wB[0:K, B_ONE191] = 1.0
    wB[0, B_ONE119:B_ONE119 + K] = 1.0
    wB[0, B_VB:B_VB + CI] = np.asarray(wts["v_b"], np.float32)

    wF = np.zeros((128, NFW), np.float32)
    wF[:, F_GANK] = ga_nk[0:128]
    wF[0:24, F_GANK + 1] = ga_nk[128:KN]
    wF[0:K, F_CAMB] = np.asarray(wts["conv_cam_b"], np.float32)
    wF[0:K, F_FB] = fb
    wF[0:K, F_RAM1] = ra - 1.0
    wF[:, F_KB:F_KB + 2] = np.asarray(wts["k_b"], np.float32).reshape(IC, 128).T
    wF[:, F_QB:F_QB + 2] = np.asarray(wts["q_b"], np.float32).reshape(IC, 128).T
    wF[:, F_GAMMA:F_GAMMA + 4] = np.asarray(wts["bn_gamma"], np.float32).reshape(CC, 128).T
    wF[:, F_BETA:F_BETA + 4] = np.asarray(wts["bn_beta"], np.float32).reshape(CC, 128).T
    wF[:, F_OAM1:F_OAM1 + 4] = (np.asarray(wts["out_a"], np.float32) - 1.0).reshape(CC, 128).T
    wF[:, F_EPS] = EPS

    bf = ml_dtypes.bfloat16
    return {
        "wpackB": wB.astype(bf), "wpackF": wF,
        "w2T": np.ascontiguousarray(np.asarray(wts["gcn_w2"], np.float32).T).astype(bf),
        "kwT": np.ascontiguousarray(np.asarray(wts["k_w"], np.float32).T).astype(bf),
        "vwT": np.ascontiguousarray(np.asarray(wts["v_w"], np.float32).T).astype(bf),
        "qwT": np.ascontiguousarray(np.asarray(wts["q_w"], np.float32).T).astype(bf),
        "outwT": np.ascontiguousarray(np.asarray(wts["out_w"], np.float32).T).astype(bf),
    }


WEIGHT_SPECS = [
    ("wpackB", [128, NBW], BF16), ("wpackF", [128, NFW], F32),
    ("w2T", [C, C], BF16), ("kwT", [C, CI], BF16), ("vwT", [C, CI], BF16),
    ("qwT", [C, CI], BF16), ("outwT", [CI, C], BF16),
]


def _load_chunked(nc, pool, ap, r, cdim, name, dt=BF16):
    """DRAM [r, cdim] (r = n*128) -> SBUF [128, n*cdim], column-grouped."""
    nchunk = r // 128
    t = pool.tile([128, nchunk * cdim], dt, name=name)
    src = ap.rearrange("(n p) c -> p n c", p=128)
    nc.sync.dma_start(t[:].rearrange("p (n c) -> p n c", n=nchunk), src)
    return t


def build_caam(tc, outs, ins, n_cores):
    nc = tc.nc
    x_d = ins["x"]
    y_d = outs["y"]
    Ntot = float(n_cores * HWp)

    # ---------------- pool stack (LIFO) ----------------
    wpool = tc.alloc_tile_pool(name="wts", bufs=1)
    dpool = tc.alloc_tile_pool(name="stats", bufs=1)
    attw = tc.alloc_tile_pool(name="attw", bufs=1)
    ypool = tc.alloc_tile_pool(name="y_res", bufs=1)
    xpool = tc.alloc_tile_pool(name="x_res", bufs=1)
    spool = tc.alloc_tile_pool(name="slot", bufs=1)
    gpool = tc.alloc_tile_pool(name="gcn", bufs=1)

    wB = wpool.tile([128, NBW], BF16, name="wpackB")
    nc.sync.dma_start(wB[:], ins["wpackB"])
    wF = wpool.tile([128, NFW], F32, name="wpackF")
    nc.sync.dma_start(wF[:], ins["wpackF"])
    qwT = _load_chunked(nc, wpool, ins["qwT"], C, CI, "qwT")
    outwT = _load_chunked(nc, wpool, ins["outwT"], CI, C, "outwT")
    idn = wB[:, B_IDN:B_IDN + 128]

    dsA = dpool.tile([128, NA], F32, name="dsA")
    dsD = dpool.tile([128, ND], F32, name="dsD")

    # x resident, bin-blocked bf16; chunk-interleaved loads so phase A can
    # start after the first wave instead of after the full 8 MB
    x_sb = [xpool.tile([128, HWp], BF16, name=f"x_{cc}") for cc in range(CC)]
    XQ = HWp // 4
    for q in range(4):
        for cc in range(CC):
            nc.sync.dma_start(x_sb[cc][:, q * XQ:(q + 1) * XQ],
                              x_d[cc * 128:(cc + 1) * 128, q * XQ:(q + 1) * XQ])
    y_sb = [ypool.tile([128, HWp], BF16, name=f"y_{cc}") for cc in range(CC)]

    camE = spool.tile([K, HWp], BF16, tag="slot", name="camE")

    # ---------------- phase A: CAM + exp + bin stats ----------------
    with tc.tile_pool(name="phA_ps", bufs=1, space="PSUM") as aps:
        for ch in range(HWp // 512):
            cp = aps.tile([K, 512], F32, tag="camps", bufs=2)
            for cc in range(CC):
                nc.tensor.matmul(cp[:], wB[:, B_WCAM + cc*K:B_WCAM + (cc+1)*K],
                                 x_sb[cc][:, ch * 512:(ch + 1) * 512],
                                 start=(cc == 0), stop=(cc == CC - 1))
            nc.scalar.activation(camE[:, ch * 512:(ch + 1) * 512], cp[:],
                                 ACT.Identity, bias=wF[0:K, F_CAMB:F_CAMB + 1],
                                 accum_out=dsA[0:K, A_CS + ch:A_CS + ch + 1])
            nc.scalar.activation(camE[:, ch * 512:(ch + 1) * 512],
                                 camE[:, ch * 512:(ch + 1) * 512], ACT.Exp,
                                 accum_out=dsA[0:K, A_ES + ch:A_ES + ch + 1])
    cs2 = dsA[0:K, A_CS:A_CS + 16].rearrange("p (n two) -> p n two", two=2)
    nc.vector.tensor_add(dsA[0:K, A_CSB:A_CSB + NB], cs2[:, :, 0], cs2[:, :, 1])
    es2 = dsA[0:K, A_ES:A_ES + 16].rearrange("p (n two) -> p n two", two=2)
    nc.vector.tensor_add(dsA[0:K, A_ESB:A_ESB + NB], es2[:, :, 0], es2[:, :, 1])
    nc.scalar.activation(dsA[0:K, A_CLS:A_CLS + NB], dsA[0:K, A_CSB:A_CSB + NB],
                         ACT.Sigmoid, scale=1.0 / P)
    nc.vector.reciprocal(dsA[0:K, A_REC:A_REC + NB], dsA[0:K, A_ESB:A_ESB + NB])
    nc.vector.tensor_mul(dsA[0:K, A_SCL:A_SCL + NB],
                         dsA[0:K, A_CLS:A_CLS + NB], dsA[0:K, A_REC:A_REC + NB])

    # ---------------- phase B: E^T and per-bin local ----------------
    # ETall[:, pc*19:(pc+1)*19] = camE[:, pc*128:(pc+1)*128]^T
    ETall = gpool.tile([128, NPC * K], BF16, name="ETall")
    stack = gpool.tile([128, 2 * C], BF16, name="stack")
    stackA = stack[:, 0:C]
    stackB = stack[0:24, C:2 * C]
    with tc.tile_pool(name="phB_sb", bufs=1) as bsb, \
         tc.tile_pool(name="phB_ps", bufs=1, space="PSUM") as bps:
        for g in range(NPC // 4):  # 4 transposes per psum tile, 1 copy
            # 20-col slots: bf16 PSUM writes must be 4-byte aligned
            tE = bps.tile([128, 4 * 20], BF16, tag="tE", bufs=2)
            for j in range(4):
                pc = g * 4 + j
                nc.tensor.transpose(tE[:, j * 20:j * 20 + K],
                                    camE[0:K, pc * 128:(pc + 1) * 128],
                                    idn[0:K, 0:K])
            dstE = ETall[:, g * 4 * K:(g + 1) * 4 * K].rearrange("p (j k) -> p j k", j=4)
            srcE = tE[:].rearrange("p (j k) -> p j k", j=4)[:, :, 0:K]
            nc.vector.tensor_copy(dstE, srcE)
        for n in range(NB):
            locp = bps.tile([K, C], F32, tag="locp", bufs=2)
            for pq in range(4):
                xps = bps.tile([128, 2 * C], BF16, tag="xps", bufs=2)
                for half in range(2):
                    p0 = n * P + (pq * 2 + half) * 128
                    for cc in range(CC):
                        nc.tensor.transpose(xps[:, half * C + cc * 128: half * C + (cc + 1) * 128],
                                            x_sb[cc][:, p0:p0 + 128], idn)
                xpp = bsb.tile([128, 2 * C], BF16, tag="xpp", bufs=3)
                if pq % 2 == 0:
                    nc.scalar.copy(xpp[:], xps[:])
                else:
                    nc.vector.tensor_copy(xpp[:], xps[:])
                for half in range(2):
                    pc = pq * 2 + half
                    nc.tensor.matmul(locp[:], ETall[:, (n * 8 + pc) * K:(n * 8 + pc + 1) * K],
                                     xpp[:, half * C:(half + 1) * C],
                                     start=(pc == 0), stop=(pc == 7))
            locS = bsb.tile([K, C], BF16, tag="locS", bufs=2)
            nc.vector.tensor_single_scalar(locS[:], locp[:],
                                           dsA[0:K, A_SCL + n:A_SCL + n + 1], OP.mult)
            # stack rows n*19 .. n*19+19 (may straddle chunks at p=128)
            p0, p1 = n * K, n * K + K
            if p1 <= 128:
                nc.sync.dma_start(stackA[p0:p1, :], locS[:, :])
            elif p0 >= 128:
                nc.sync.dma_start(stackB[p0 - 128:p1 - 128, :], locS[:, :])
            else:
                nc.sync.dma_start(stackA[p0:128, :], locS[0:128 - p0, :])
                nc.sync.dma_start(stackB[0:p1 - 128, :], locS[128 - p0:K, :])

    # ---------------- phase C: GCN + fuse + key/val ----------------
    atp = attw.tile([128, NAT], BF16, name="attpack")
    keyT = atp[:, AT_KEYT:AT_KEYT + IC * KN]
    val = atp[0:K, AT_VAL:AT_VAL + CI]
    scr = spool.tile([128, NS], BF16, tag="slot", name="scr")
    vA = scr[:, S_VA:S_VA + C]
    vB = scr[0:24, S_VB:S_VB + C]
    with tc.tile_pool(name="phC_sb", bufs=1) as csb, \
         tc.tile_pool(name="phC_ps", bufs=1, space="PSUM") as cps:
        w2T = _load_chunked(nc, csb, ins["w2T"], C, C, "w2T")
        kwT = _load_chunked(nc, csb, ins["kwT"], C, CI, "kwT")
        vwT = _load_chunked(nc, csb, ins["vwT"], C, CI, "vwT")
        # conv1: t = W1NK.T @ stack (contraction over 152 stack rows)
        tpA = cps.tile([128, C], F32, tag="big")
        nc.tensor.matmul(tpA[:], wB[:, B_W1NK0:B_W1NK0 + 128], stackA, start=True, stop=False)
        nc.tensor.matmul(tpA[:], wB[0:24, B_W1NK1:B_W1NK1 + 128], stackB, start=False, stop=True)
        tpB = cps.tile([24, C], F32, tag="smallB")
        nc.tensor.matmul(tpB[:], wB[:, B_W1NK0 + 128:B_W1NK0 + KN], stackA, start=True, stop=False)
        nc.tensor.matmul(tpB[:], wB[0:24, B_W1NK1 + 128:B_W1NK1 + KN], stackB, start=False, stop=True)
        # prelu(t + stack), per-row alpha gcn_a[n] (F_GANK cols)
        for (tp, st, vv, gchunk, rows) in ((tpA, stackA, vA, 0, 128),
                                           (tpB, stackB, vB, 1, 24)):
            u_ = scr[0:rows, S_UG:S_UG + C]
            nc.vector.tensor_add(u_, tp[:], st)
            m_ = scr[0:rows, S_MG:S_MG + C]
            nc.gpsimd.tensor_scalar_min(m_, u_, 0.0)
            nc.vector.scalar_tensor_tensor(vv, m_, wF[0:rows, F_GANK + gchunk:F_GANK + gchunk + 1],
                                           u_, OP.mult, OP.add)
        # transpose t -> tT [c, (n,k)]
        for cc in range(CC):
            tt = scr[:, S_TT + cc * KN:S_TT + (cc + 1) * KN]
            pA = cps.tile([128, KN], BF16, tag="tr", bufs=2)
            nc.tensor.transpose(pA[:, 0:128], vA[:, cc * 128:(cc + 1) * 128], idn)
            nc.tensor.transpose(pA[:, 128:KN], vB[:, cc * 128:(cc + 1) * 128], idn[0:24, 0:24])
            nc.scalar.copy(tt[:], pA[:])
        # w2: local2 = t @ w2T (stack layout out)
        l2A = scr[:, S_L2A:S_L2A + C]
        l2B = scr[0:24, S_L2B:S_L2B + C]
        pl2A = cps.tile([128, C], F32, tag="big")
        for cc in range(CC):
            nc.tensor.matmul(pl2A[:], scr[:, S_TT + cc * KN:S_TT + cc * KN + 128],
                             w2T[:, cc * C:(cc + 1) * C], start=(cc == 0), stop=(cc == CC - 1))
        nc.scalar.copy(l2A, pl2A[:])
        pl2B = cps.tile([24, C], F32, tag="smallB")
        for cc in range(CC):
            nc.tensor.matmul(pl2B[:], scr[:, S_TT + cc * KN + 128:S_TT + cc * KN + 152],
                             w2T[:, cc * C:(cc + 1) * C], start=(cc == 0), stop=(cc == CC - 1))
        nc.scalar.copy(l2B, pl2B[:])
        # fuse -> glob [19, 512], then prelu
        gp = cps.tile([K, C], F32, tag="gAB")
        nc.tensor.matmul(gp[:], wB[:, B_FNK0:B_FNK0 + K], l2A, start=True, stop=False)
        nc.tensor.matmul(gp[:], wB[0:24, B_FNK1:B_FNK1 + K], l2B, start=False, stop=True)
        glob = scr[0:K, S_GL:S_GL + C]
        u_ = scr[0:K, S_UG2:S_UG2 + C]
        nc.vector.tensor_scalar_add(u_, gp[:], wF[0:K, F_FB:F_FB + 1])
        m_ = scr[0:K, S_MG2:S_MG2 + C]
        nc.gpsimd.tensor_scalar_min(m_, u_, 0.0)
        nc.vector.scalar_tensor_tensor(glob, m_, wF[0:K, F_RAM1:F_RAM1 + 1], u_, OP.mult, OP.add)
        # globT + val (+ v_b via ones-row matmul)
        valp = cps.tile([K, CI], F32, tag="gAB")
        for cc in range(CC):
            gt = scr[:, S_GT + cc * K:S_GT + (cc + 1) * K]
            pA = cps.tile([128, K], BF16, tag="tr", bufs=2)
            nc.tensor.transpose(pA[:], glob[:, cc * 128:(cc + 1) * 128], idn[0:K, 0:K])
            nc.scalar.copy(gt[:, :], pA[:])
            nc.tensor.matmul(valp[:], gt[:], vwT[:, cc * CI:(cc + 1) * CI],
                             start=(cc == 0), stop=False)
        nc.tensor.matmul(valp[:], wB[0:1, B_ONE119:B_ONE119 + K], wB[0:1, B_VB:B_VB + CI],
                         start=False, stop=True)
        nc.scalar.copy(val, valp[:])
        # local2T + keyT (+ k_b per-partition bias)
        for cc in range(CC):
            lt = scr[:, S_L2T + cc * KN:S_L2T + (cc + 1) * KN]
            pA = cps.tile([128, KN], BF16, tag="tr", bufs=2)
            nc.tensor.transpose(pA[:, 0:128], l2A[:, cc * 128:(cc + 1) * 128], idn)
            nc.tensor.transpose(pA[:, 128:KN], l2B[:, cc * 128:(cc + 1) * 128], idn[0:24, 0:24])
            nc.vector.tensor_copy(lt[:], pA[:])
        for ic in range(IC):
            kp = cps.tile([128, KN], F32, tag="keyp", bufs=1)
            for cc in range(CC):
                nc.tensor.matmul(kp[:], kwT[:, cc * CI + ic * 128: cc * CI + (ic + 1) * 128],
                                 scr[:, S_L2T + cc * KN:S_L2T + (cc + 1) * KN],
                                 start=(cc == 0), stop=(cc == CC - 1))
            nc.scalar.activation(keyT[:, ic * KN:(ic + 1) * KN], kp[:], ACT.Identity,
                                 bias=wF[:, F_KB + ic:F_KB + ic + 1])
    gpool.release()

    # ---------------- phase D: attention + y (single pass) ----------------
    with tc.tile_pool(name="phD_sb", bufs=1) as dsb, \
         tc.tile_pool(name="phD_ps", bufs=1, space="PSUM") as dps:
        for n in range(NB):
            qT = dsb.tile([128, IC * P], BF16, tag="qT", bufs=2)
            for ic in range(IC):
                for nh in range(2):
                    qp = dps.tile([128, 512], F32, tag="qp", bufs=2)
                    for cc in range(CC):
                        xsl = x_sb[cc][:, n * P + nh * 512: n * P + (nh + 1) * 512]
                        nc.tensor.matmul(qp[:], qwT[:, cc * CI + ic * 128: cc * CI + (ic + 1) * 128],
                                         xsl, start=(cc == 0), stop=(cc == CC - 1))
                    qsl = qT[:, ic * P + nh * 512: ic * P + (nh + 1) * 512]
                    if nh == 0:
                        nc.scalar.activation(qsl, qp[:], ACT.Identity,
                                             bias=wF[:, F_QB + ic:F_QB + ic + 1])
                    else:
                        nc.vector.tensor_scalar_add(qsl, qp[:],
                                                    wF[:, F_QB + ic:F_QB + ic + 1])
            ehat = dsb.tile([K, P], BF16, tag="ehat", bufs=2)
            sp = dps.tile([1, P], F32, tag="aop", bufs=1)
            for nh in range(2):
                afp = dps.tile([K, 512], F32, tag="soft", bufs=2)
                for ic in range(IC):
                    ksel = keyT[:, ic * KN + n * K: ic * KN + (n + 1) * K]
                    nc.tensor.matmul(afp[:], ksel, qT[:, ic * P + nh * 512: ic * P + (nh + 1) * 512],
                                     start=(ic == 0), stop=(ic == IC - 1))
                esl = ehat[:, nh * 512:(nh + 1) * 512]
                nc.scalar.activation(esl, afp[:], ACT.Exp)
                nc.tensor.matmul(sp[:, nh * 512:(nh + 1) * 512],
                                 wB[0:K, B_ONE191:B_ONE191 + 1], esl,
                                 start=True, stop=True)
            rrow = dsD[0:1, D_RROW:D_RROW + P]
            nc.vector.reciprocal(rrow, sp[:])
            rrowB = dsb.tile([1, P], BF16, tag="rrowB", bufs=2)
            nc.vector.tensor_copy(rrowB[:], rrow)
            for nh in range(2):
                rbp = dps.tile([K, 512], F32, tag="soft", bufs=2)
                nc.tensor.matmul(rbp[:], wB[0:1, B_ONE119:B_ONE119 + K],
                                 rrowB[:, nh * 512:(nh + 1) * 512],
                                 start=True, stop=True)
                nc.vector.tensor_mul(ehat[:, nh * 512:(nh + 1) * 512],
                                     ehat[:, nh * 512:(nh + 1) * 512], rbp[:])
            at = []
            for ic in range(IC):
                a_ = dsb.tile([128, P], BF16, tag=f"at{ic}", bufs=2)
                aop = dps.tile([128, P], F32, tag="aop", bufs=1)
                for nh in range(2):
                    nc.tensor.matmul(aop[:, nh * 512:(nh + 1) * 512],
                                     val[:, ic * 128:(ic + 1) * 128],
                                     ehat[:, nh * 512:(nh + 1) * 512], start=True, stop=True)
                # attn row-sums ride the copy: mu comes from outw @ sum(attn)
                rcol = dsD[:, D_YS + ic * NB + n: D_YS + ic * NB + n + 1]
                nc.scalar.activation(a_[:], aop[:], ACT.Copy, accum_out=rcol)
                at.append(a_)
            for cc in range(CC):
                for nh in range(2):
                    yp = dps.tile([128, 512], F32, tag="yp", bufs=2)
                    for ic in range(IC):
                        nc.tensor.matmul(yp[:], outwT[:, ic * C + cc * 128: ic * C + (cc + 1) * 128],
                                         at[ic][:, nh * 512:(nh + 1) * 512],
                                         start=(ic == 0), stop=(ic == IC - 1))
                    ysl = y_sb[cc][:, n * P + nh * 512: n * P + (nh + 1) * 512]
                    nc.vector.tensor_copy(ysl, yp[:])
                    sq = dsb.tile([128, 512], BF16, tag="ysq", bufs=2)
                    col2 = D_SQ + cc * 16 + n * 2 + nh
                    nc.scalar.activation(sq[:], yp[:], ACT.Square,
                                         accum_out=dsD[:, col2:col2 + 1])
        # mu: Sigma_y per cc = outwT @ (per-ic attn row sums, summed over bins)
        rs2 = dsD[:, D_YS:D_YS + 2 * NB].rearrange("p (i b) -> p i b", i=IC)
        nc.vector.tensor_reduce(dsD[:, D_YS + 16:D_YS + 16 + IC], rs2, axis=AX.X, op=OP.add)
        rsB = dsb.tile([128, IC], BF16, tag="rsB")
        nc.vector.tensor_copy(rsB[:], dsD[:, D_YS + 16:D_YS + 16 + IC])
        for cc in range(CC):
            mup = dps.tile([128, 1], F32, tag="yp", bufs=2)
            for ic in range(IC):
                nc.tensor.matmul(mup[:], outwT[:, ic * C + cc * 128: ic * C + (cc + 1) * 128],
                                 rsB[:, ic:ic + 1],
                                 start=(ic == 0), stop=(ic == IC - 1))
            nc.vector.tensor_copy(dsD[:, D_ST + 2 * cc:D_ST + 2 * cc + 1], mup[:])
    sqsum = dsD[:, D_SQ:D_SQ + 64].rearrange("p (c b) -> p c b", c=CC)
    st2 = dsD[:, D_ST:D_ST + 2 * CC].rearrange("p (c two) -> p c two", two=2)
    nc.vector.tensor_reduce(st2[:, :, 1], sqsum, axis=AX.X, op=OP.add)

    # ---------------- collective ----------------
    with tc.tile_pool(name="cdram", bufs=1, space="DRAM") as cdram:
        arin = cdram.tile([128, 2 * CC], F32)
        arout = cdram.tile([128, 2 * CC], F32)
        nc.sync.dma_start(arin[:], dsD[:, D_ST:D_ST + 2 * CC])
        nc.gpsimd.collective_compute(
            "AllReduce", OP.add,
            ins=[arin.opt()], outs=[arout.opt()],
            replica_groups=[list(range(n_cores))],
        )
        nc.sync.dma_start(dsD[:, D_SBN:D_SBN + 2 * CC], arout[:])

    # ---------------- BN finalize ----------------
    mom = dsD[:, D_MOM:D_MOM + 2 * CC]
    nc.scalar.mul(mom, dsD[:, D_SBN:D_SBN + 2 * CC], 1.0 / Ntot)
    muv = mom.rearrange("p (c two) -> p c two", two=2)[:, :, 0]
    msq = mom.rearrange("p (c two) -> p c two", two=2)[:, :, 1]
    nc.vector.tensor_mul(dsD[:, D_MUSQ:D_MUSQ + CC], muv, muv)
    nc.vector.tensor_sub(dsD[:, D_VAR:D_VAR + CC], msq, dsD[:, D_MUSQ:D_MUSQ + CC])
    nc.scalar.activation(dsD[:, D_SD:D_SD + CC], dsD[:, D_VAR:D_VAR + CC], ACT.Sqrt,
                         bias=wF[:, F_EPS:F_EPS + 1])
    nc.vector.reciprocal(dsD[:, D_RSTD:D_RSTD + CC], dsD[:, D_SD:D_SD + CC])
    scol = dsD[:, D_SCOL:D_SCOL + CC]
    bcol = dsD[:, D_BCOL:D_BCOL + CC]
    nc.vector.tensor_mul(scol, wF[:, F_GAMMA:F_GAMMA + CC], dsD[:, D_RSTD:D_RSTD + CC])
    nc.vector.tensor_scalar_mul(dsD[:, D_NSC:D_NSC + CC], scol, -1.0)
    for cc in range(CC):
        nc.vector.scalar_tensor_tensor(bcol[:, cc:cc + 1], muv[:, cc:cc + 1],
                                       dsD[:, D_NSC + cc:D_NSC + cc + 1],
                                       wF[:, F_BETA + cc:F_BETA + cc + 1], OP.mult, OP.add)

    # ---------------- phase F: scale/shift + PReLU + residual ----------------
    yv = y_d.rearrange("c h w -> c (h w)")
    stage2 = spool.tile([128, RH * W], BF16, name="stage2")
    with tc.tile_pool(name="phF_sb", bufs=1) as fsb:
        stage1 = spool.tile([128, RH * W], BF16, tag="slot", name="stage1")
        stages = [stage1, stage2]
        for bi in range(BH):
            for cc in range(CC):
                st_ = stages[(bi * CC + cc) % 2]
                stv = st_[:].rearrange("p (h w) -> p h w", w=W)
                for bj in range(BW):
                    n = bi * BW + bj
                    ysl = y_sb[cc][:, n * P:(n + 1) * P]
                    u = fsb.tile([128, P], BF16, tag="u_f", bufs=3)
                    nc.scalar.activation(u[:], ysl, ACT.Identity,
                                         bias=bcol[:, cc:cc + 1], scale=scol[:, cc:cc + 1])
                    m2 = fsb.tile([128, P], BF16, tag="m_f", bufs=3)
                    nc.gpsimd.tensor_scalar_min(m2[:], u[:], 0.0)
                    xpu = fsb.tile([128, P], BF16, tag="xpu_f", bufs=3)
                    nc.vector.scalar_tensor_tensor(xpu[:], m2[:],
                                                   wF[:, F_OAM1 + cc:F_OAM1 + cc + 1],
                                                   u[:], OP.mult, OP.add)
                    dst = stv[:, :, RW * bj:RW * (bj + 1)]
                    xres = x_sb[cc][:, n * P:(n + 1) * P]
                    if bj % 2 == 0:
                        nc.vector.tensor_add(dst, xpu[:], xres)
                    else:
                        nc.gpsimd.tensor_add(dst, xpu[:], xres)
                nc.sync.dma_start(yv[cc * 128:(cc + 1) * 128, RH * bi * W:RH * (bi + 1) * W],
                                  st_[:])
    spool.release()
    xpool.release()
    ypool.release()
    attw.release()
    dpool.release()
    wpool.release()


# ======================================================================
# Entry point: kernel(**inputs) -> np.ndarray [8, 512, 64, 128]
# ======================================================================
import concourse.bacc as bacc
import concourse.tile as tile
from concourse.bass_utils import run_bass_kernel_spmd

N_CORES = 8
_cached = {}


def _build_program(n_cores=N_CORES):
    if "nc" in _cached:
        return _cached["nc"]
    nc = bacc.Bacc("TRN2", target_bir_lowering=False, debug=False, num_devices=n_cores)
    ins = {"x": nc.dram_tensor("x", [C, HWp], BF16, kind="ExternalInput").ap()}
    for nm, shape, dt in WEIGHT_SPECS:
        ins[nm] = nc.dram_tensor(nm, shape, dt, kind="ExternalInput").ap()
    outs = {"y": nc.dram_tensor("y", [C, H, W], BF16, kind="ExternalOutput").ap()}
    with tile.TileContext(nc) as tc:
        build_caam(tc, outs, ins, n_cores)
    nc.compile()
    _cached["nc"] = nc
    return nc


def make_in_maps(inputs):
    x = np.ascontiguousarray(np.asarray(inputs["x"], np.float32))
    prep = host_prep(inputs)
    bf = ml_dtypes.bfloat16
    in_maps = []
    for c in range(N_CORES):
        # bin-blocked: [C, H, W] -> [C, bi, ph, bj, pw] -> [C, (bi bj ph pw)]
        xb = x[c].reshape(C, BH, RH, BW, RW).transpose(0, 1, 3, 2, 4)
        d = {"x": np.ascontiguousarray(xb.reshape(C, HWp)).astype(bf)}
        for nm, _, _ in WEIGHT_SPECS:
            d[nm] = prep[nm]
        in_maps.append(d)
    return in_maps


def kernel(**inputs):
    nc = _build_program()
    in_maps = make_in_maps(inputs)
    res = run_bass_kernel_spmd(nc, in_maps, core_ids=list(range(N_CORES)))
    return np.stack([res.results[c]["y"] for c in range(N_CORES)]).astype(np.float32)


# revision 26
# speedup vs baseline: 2.2180x; 2.2180x over previous
"""CAAM kernel for Trainium2: bf16 single-pass design.

Per-core: one batch element. x arrives host-permuted to bin-blocked
layout [C, 8192] bf16 (free index = bin*1024 + ph*32 + pw). Layouts:
  x resident:  4 SBUF tiles [128, 8192] bf16
  y resident:  4 SBUF tiles [128, 8192] bf16 (attention output, pre-BN)
  camE [19, 8192] bf16: cam, then exp(cam) in place (shares slot with
  phase-C scratch and one phase-F stage buffer)
  ETall [128, 64*19] bf16: exp(cam) transposed per 128-pixel chunk
  stack [(k*8+n), c] bf16 (GCN input), chunk0 rows 0:128, chunk1 0:24
  keyT [128i, 2*152] + val [19, 256] bf16
  per bin: qT [128, 2*1024] bf16, Ehat [19, 1024] bf16 (normalized aff),
  attnT 2x [128, 1024] bf16, y psum -> y resident + sum/sumsq riders.
  BN stats allreduced; phase F: scale/shift + PReLU + residual from
  resident y/x, staged per (bi, cc) for contiguous output DMA.
"""

import numpy as np
import ml_dtypes
import concourse.bass as bass
import concourse.mybir as mybir

F32 = mybir.dt.float32
BF16 = mybir.dt.bfloat16
AX = mybir.AxisListType
OP = mybir.AluOpType
ACT = mybir.ActivationFunctionType

B, C, H, W = 8, 512, 64, 128
K, BH, BW = 19, 2, 4
NB = BH * BW          # 8
CI = C // 2           # 256
HWp = H * W           # 8192
RH, RW = H // BH, W // BW   # 32, 32
P = RH * RW           # 1024
CC = C // 128         # 4
IC = CI // 128        # 2
KN = K * NB           # 152
NPC = HWp // 128      # 64 pixel chunks
EPS = 1e-5

# -------- wpackB column map (bf16 consts) --------
B_IDN = 0        # 128 cols            identity
B_WCAM = 128     # 76 = 4 x 19         conv_cam lhsT chunks
B_W1NK0 = 204    # 152, rows 0:128     gcn conv1 lhsT chunk0
B_W1NK1 = 356    # 152, rows 0:24      chunk1
B_FNK0 = 508     # 19, rows 0:128      fuse lhsT chunk0
B_FNK1 = 527     # 19, rows 0:24       chunk1
B_ONE191 = 546   # 1 col, rows 0:19    ones (sum-over-k lhsT)
B_ONE119 = 547   # 19 cols, row 0      ones (broadcast lhsT)
B_VB = 566       # 256 cols, row 0     v_b
NBW = 822

# -------- wpackF column map (fp32 consts, [128, 22]) --------
F_GANK = 0       # 2: gcn_a-1 per stack row (chunk0, chunk1)
F_CAMB = 2       # 1, rows 0:19
F_FB = 3         # 1, rows 0:19  fuse_b
F_RAM1 = 4       # 1, rows 0:19  relu_a - 1
F_KB = 5         # 2             k_b chunks
F_QB = 7         # 2             q_b chunks
F_GAMMA = 9      # 4
F_BETA = 13      # 4
F_OAM1 = 17      # 4  out_a - 1
F_EPS = 21       # 1
NFW = 22

# -------- dsA ([128, 72]): phase A stats (rows 0:19) --------
A_CS = 0         # 16: cam sums per 512-chunk
A_ES = 16        # 16: exp sums per 512-chunk
A_CSB = 32       # 8: cam sums per bin
A_ESB = 40       # 8: exp sums per bin
A_CLS = 48       # 8: sigmoid(mean cam)
A_REC = 56       # 8: 1/esum
A_SCL = 64       # 8: cls * rec
NA = 72

# -------- dsD ([128, 704]) --------
D_YS = 0         # 64: y sums per (cc, bin, nh)
D_SQ = 64        # 64: y^2 sums per (cc, bin, nh)
D_ST = 128       # 8: packed allreduce input (sum, sumsq per cc)
D_SBN = 136      # 8: allreduce output
D_MOM = 144      # 8
D_VAR = 152      # 4
D_MUSQ = 156     # 4
D_SD = 160       # 4
D_RSTD = 164     # 4
D_SCOL = 168     # 4
D_BCOL = 172     # 4
D_NSC = 176      # 4
D_RROW = 192     # 1024, row 0
ND = 1216

# -------- scr column map (phase-C scratch, bf16, shares camE slot) ----
S_VA = 0         # 512   prelu'd t, chunk0
S_VB = 512       # 512, rows 0:24  chunk1
S_UG = 1024      # 512   u scratch (fp32 would be better but bf16 ok)
S_MG = 1536      # 512
S_TT = 2048      # 608 = 4 x 152   t transposed
S_L2A = 2656     # 512
S_L2B = 3168     # 512, rows 0:24
S_GL = 3680      # 512, rows 0:19  glob (prelu'd)
S_UG2 = 4192     # 512
S_MG2 = 4704     # 512
S_GT = 5216      # 76 = 4 x 19     glob transposed
S_L2T = 5292     # 608             local2 transposed
NS = 5900

# attw pack: keyT 0:304, val 304:560
AT_KEYT = 0
AT_VAL = 304
NAT = 560


def host_prep(wts: dict) -> dict:
    w1 = np.asarray(wts["gcn_w1"], np.float32)
    ga = np.asarray(wts["gcn_a"], np.float32)
    fw = np.asarray(wts["fuse_w"], np.float32).reshape(-1)
    fb = float(np.asarray(wts["fuse_b"], np.float32).reshape(-1)[0])
    ra = float(np.asarray(wts["relu_a"], np.float32).reshape(-1)[0])

    wB = np.zeros((128, NBW), np.float32)
    wB[:, B_IDN:B_IDN + 128] = np.eye(128, dtype=np.float32)
    wcamT = np.asarray(wts["conv_cam_w"], np.float32).T    # [512, 19]
    for cc in range(CC):
        wB[:, B_WCAM + cc*K:B_WCAM + (cc+1)*K] = wcamT[cc*128:(cc+1)*128]
    # conv1 lhsT: W[(m*19+kp), (n*19+k)] = w1[n, m] * (kp == k)
    W1NK = np.zeros((KN, KN), np.float32)
    FNK = np.zeros((KN, K), np.float32)
    ga_nk = np.zeros(KN, np.float32)
    for n in range(NB):
        for k in range(K):
            for m in range(NB):
                W1NK[m*K + k, n*K + k] = w1[n, m]
            FNK[n*K + k, k] = fw[n]
            ga_nk[n*K + k] = ga[n] - 1.0
    wB[:, B_W1NK0:B_W1NK0 + KN] = W1NK[0:128]
    wB[0:24, B_W1NK1:B_W1NK1 + KN] = W1NK[128:KN]
    wB[:, B_FNK0:B_FNK0 + K] = FNK[0:128]
    wB[0:24, B_FNK1:B_FNK1 + K] = FNK[128:KN]
    wB[0:K, B_ONE191] = 1.0
    wB[0, B_ONE119:B_ONE119 + K] = 1.0
    wB[0, B_VB:B_VB + CI] = np.asarray(wts["v_b"], np.float32)

    wF = np.zeros((128, NFW), np.float32)
    wF[:, F_GANK] = ga_nk[0:128]
    wF[0:24, F_GANK + 1] = ga_nk[128:KN]
    wF[0:K, F_CAMB] = np.asarray(wts["conv_cam_b"], np.float32)
    wF[0:K, F_FB] = fb
    wF[0:K, F_RAM1] = ra - 1.0
    wF[:, F_KB:F_KB + 2] = np.asarray(wts["k_b"], np.float32).reshape(IC, 128).T
    wF[:, F_QB:F_QB + 2] = np.asarray(wts["q_b"], np.float32).reshape(IC, 128).T
    wF[:, F_GAMMA:F_GAMMA + 4] = np.asarray(wts["bn_gamma"], np.float32).reshape(CC, 128).T
    wF[:, F_BETA:F_BETA + 4] = np.asarray(wts["bn_beta"], np.float32).reshape(CC, 128).T
    wF[:, F_OAM1:F_OAM1 + 4] = (np.asarray(wts["out_a"], np.float32) - 1.0).reshape(CC, 128).T
    wF[:, F_EPS] = EPS

    bf = ml_dtypes.bfloat16
    return {
        "wpackB": wB.astype(bf), "wpackF": wF,
        "w2T": np.ascontiguousarray(np.asarray(wts["gcn_w2"], np.float32).T).astype(bf),
        "kwT": np.ascontiguousarray(np.asarray(wts["k_w"], np.float32).T).astype(bf),
        "vwT": np.ascontiguousarray(np.asarray(wts["v_w"], np.float32).T).astype(bf),
        "qwT": np.ascontiguousarray(np.asarray(wts["q_w"], np.float32).T).astype(bf),
        "outwT": np.ascontiguousarray(np.asarray(wts["out_w"], np.float32).T).astype(bf),
    }


WEIGHT_SPECS = [
    ("wpackB", [128, NBW], BF16), ("wpackF", [128, NFW], F32),
    ("w2T", [C, C], BF16), ("kwT", [C, CI], BF16), ("vwT", [C, CI], BF16),
    ("qwT", [C, CI], BF16), ("outwT", [CI, C], BF16),
]


def _load_chunked(nc, pool, ap, r, cdim, name, dt=BF16):
    """DRAM [r, cdim] (r = n*128) -> SBUF [128, n*cdim], column-grouped."""
    nchunk = r // 128
    t = pool.tile([128, nchunk * cdim], dt, name=name)
    src = ap.rearrange("(n p) c -> p n c", p=128)
    nc.sync.dma_start(t[:].rearrange("p (n c) -> p n c", n=nchunk), src)
    return t


def build_caam(tc, outs, ins, n_cores):
    nc = tc.nc
    x_d = ins["x"]
    y_d = outs["y"]
    Ntot = float(n_cores * HWp)

    # ---------------- pool stack (LIFO) ----------------
    wpool = tc.alloc_tile_pool(name="wts", bufs=1)
    dpool = tc.alloc_tile_pool(name="stats", bufs=1)
    attw = tc.alloc_tile_pool(name="attw", bufs=1)
    ypool = tc.alloc_tile_pool(name="y_res", bufs=1)
    xpool = tc.alloc_tile_pool(name="x_res", bufs=1)
    spool = tc.alloc_tile_pool(name="slot", bufs=1)
    gpool = tc.alloc_tile_pool(name="gcn", bufs=1)

    wB = wpool.tile([128, NBW], BF16, name="wpackB")
    nc.sync.dma_start(wB[:], ins["wpackB"])
    wF = wpool.tile([128, NFW], F32, name="wpackF")
    nc.sync.dma_start(wF[:], ins["wpackF"])
    qwT = _load_chunked(nc, wpool, ins["qwT"], C, CI, "qwT")
    outwT = _load_chunked(nc, wpool, ins["outwT"], CI, C, "outwT")
    idn = wB[:, B_IDN:B_IDN + 128]

    dsA = dpool.tile([128, NA], F32, name="dsA")
    dsD = dpool.tile([128, ND], F32, name="dsD")

    # x resident, bin-blocked bf16; chunk-interleaved loads so phase A can
    # start after the first wave instead of after the full 8 MB
    x_sb = [xpool.tile([128, HWp], BF16, name=f"x_{cc}") for cc in range(CC)]
    XQ = HWp // 4
    for q in range(4):
        for cc in range(CC):
            nc.sync.dma_start(x_sb[cc][:, q * XQ:(q + 1) * XQ],
                              x_d[cc * 128:(cc + 1) * 128, q * XQ:(q + 1) * XQ])
    y_sb = [ypool.tile([128, HWp], BF16, name=f"y_{cc}") for cc in range(CC)]

    camE = spool.tile([K, HWp], BF16, tag="slot", name="camE")

    # ---------------- phase A: CAM + exp + bin stats ----------------
    with tc.tile_pool(name="phA_ps", bufs=1, space="PSUM") as aps:
        for ch in range(HWp // 512):
            cp = aps.tile([K, 512], F32, tag="camps", bufs=2)
            for cc in range(CC):
                nc.tensor.matmul(cp[:], wB[:, B_WCAM + cc*K:B_WCAM + (cc+1)*K],
                                 x_sb[cc][:, ch * 512:(ch + 1) * 512],
                                 start=(cc == 0), stop=(cc == CC - 1))
            nc.scalar.activation(camE[:, ch * 512:(ch + 1) * 512], cp[:],
                                 ACT.Identity, bias=wF[0:K, F_CAMB:F_CAMB + 1],
                                 accum_out=dsA[0:K, A_CS + ch:A_CS + ch + 1])
            nc.scalar.activation(camE[:, ch * 512:(ch + 1) * 512],
                                 camE[:, ch * 512:(ch + 1) * 512], ACT.Exp,
                                 accum_out=dsA[0:K, A_ES + ch:A_ES + ch + 1])
    cs2 = dsA[0:K, A_CS:A_CS + 16].rearrange("p (n two) -> p n two", two=2)
    nc.vector.tensor_add(dsA[0:K, A_CSB:A_CSB + NB], cs2[:, :, 0], cs2[:, :, 1])
    es2 = dsA[0:K, A_ES:A_ES + 16].rearrange("p (n two) -> p n two", two=2)
    nc.vector.tensor_add(dsA[0:K, A_ESB:A_ESB + NB], es2[:, :, 0], es2[:, :, 1])
    nc.scalar.activation(dsA[0:K, A_CLS:A_CLS + NB], dsA[0:K, A_CSB:A_CSB + NB],
                         ACT.Sigmoid, scale=1.0 / P)
    nc.vector.reciprocal(dsA[0:K, A_REC:A_REC + NB], dsA[0:K, A_ESB:A_ESB + NB])
    nc.vector.tensor_mul(dsA[0:K, A_SCL:A_SCL + NB],
                         dsA[0:K, A_CLS:A_CLS + NB], dsA[0:K, A_REC:A_REC + NB])

    # ---------------- phase B: E^T and per-bin local ----------------
    # ETall[:, pc*19:(pc+1)*19] = camE[:, pc*128:(pc+1)*128]^T
    ETall = gpool.tile([128, NPC * K], BF16, name="ETall")
    stack = gpool.tile([128, 2 * C], BF16, name="stack")
    stackA = stack[:, 0:C]
    stackB = stack[0:24, C:2 * C]
    with tc.tile_pool(name="phB_sb", bufs=1) as bsb, \
         tc.tile_pool(name="phB_ps", bufs=1, space="PSUM") as bps:
        for g in range(NPC // 4):  # 4 transposes per psum tile, 1 copy
            # 20-col slots: bf16 PSUM writes must be 4-byte aligned
            tE = bps.tile([128, 4 * 20], BF16, tag="tE", bufs=2)
            for j in range(4):
                pc = g * 4 + j
                nc.tensor.transpose(tE[:, j * 20:j * 20 + K],
                                    camE[0:K, pc * 128:(pc + 1) * 128],
                                    idn[0:K, 0:K])
            dstE = ETall[:, g * 4 * K:(g + 1) * 4 * K].rearrange("p (j k) -> p j k", j=4)
            srcE = tE[:].rearrange("p (j k) -> p j k", j=4)[:, :, 0:K]
            nc.vector.tensor_copy(dstE, srcE)
        for n in range(NB):
            locp = bps.tile([K, C], F32, tag="locp", bufs=2)
            for pq in range(4):
                xps = bps.tile([128, 2 * C], BF16, tag="xps", bufs=2)
                for half in range(2):
                    p0 = n * P + (pq * 2 + half) * 128
                    for cc in range(CC):
                        nc.tensor.transpose(xps[:, half * C + cc * 128: half * C + (cc + 1) * 128],
                                            x_sb[cc][:, p0:p0 + 128], idn)
                xpp = bsb.tile([128, 2 * C], BF16, tag="xpp", bufs=3)
                if pq % 2 == 0:
                    nc.scalar.copy(xpp[:], xps[:])
                else:
                    nc.vector.tensor_copy(xpp[:], xps[:])
                for half in range(2):
                    pc = pq * 2 + half
                    nc.tensor.matmul(locp[:], ETall[:, (n * 8 + pc) * K:(n * 8 + pc + 1) * K],
                                     xpp[:, half * C:(half + 1) * C],
                                     start=(pc == 0), stop=(pc == 7))
            locS = bsb.tile([K, C], BF16, tag="locS", bufs=2)
            nc.vector.tensor_single_scalar(locS[:], locp[:],
                                           dsA[0:K, A_SCL + n:A_SCL + n + 1], OP.mult)
            # stack rows n*19 .. n*19+19 (may straddle chunks at p=128)
            p0, p1 = n * K, n * K + K
            if p1 <= 128:
                nc.sync.dma_start(stackA[p0:p1, :], locS[:, :])
            elif p0 >= 128:
                nc.sync.dma_start(stackB[p0 - 128:p1 - 128, :], locS[:, :])
            else:
                nc.sync.dma_start(stackA[p0:128, :], locS[0:128 - p0, :])
                nc.sync.dma_start(stackB[0:p1 - 128, :], locS[128 - p0:K, :])

    # ---------------- phase C: GCN + fuse + key/val ----------------
    atp = attw.tile([128, NAT], BF16, name="attpack")
    keyT = atp[:, AT_KEYT:AT_KEYT + IC * KN]
    val = atp[0:K, AT_VAL:AT_VAL + CI]
    scr = spool.tile([128, NS], BF16, tag="slot", name="scr")
    vA = scr[:, S_VA:S_VA + C]
    vB = scr[0:24, S_VB:S_VB + C]
    with tc.tile_pool(name="phC_sb", bufs=1) as csb, \
         tc.tile_pool(name="phC_ps", bufs=1, space="PSUM") as cps:
        w2T = _load_chunked(nc, csb, ins["w2T"], C, C, "w2T")
        kwT = _load_chunked(nc, csb, ins["kwT"], C, CI, "kwT")
        vwT = _load_chunked(nc, csb, ins["vwT"], C, CI, "vwT")
        # conv1: t = W1NK.T @ stack (contraction over 152 stack rows)
        tpA = cps.tile([128, C], F32, tag="big")
        nc.tensor.matmul(tpA[:], wB[:, B_W1NK0:B_W1NK0 + 128], stackA, start=True, stop=False)
        nc.tensor.matmul(tpA[:], wB[0:24, B_W1NK1:B_W1NK1 + 128], stackB, start=False, stop=True)
        tpB = cps.tile([24, C], F32, tag="smallB")
        nc.tensor.matmul(tpB[:], wB[:, B_W1NK0 + 128:B_W1NK0 + KN], stackA, start=True, stop=False)
        nc.tensor.matmul(tpB[:], wB[0:24, B_W1NK1 + 128:B_W1NK1 + KN], stackB, start=False, stop=True)
        # prelu(t + stack), per-row alpha gcn_a[n] (F_GANK cols)
        for (tp, st, vv, gchunk, rows) in ((tpA, stackA, vA, 0, 128),
                                           (tpB, stackB, vB, 1, 24)):
            u_ = scr[0:rows, S_UG:S_UG + C]
            nc.vector.tensor_add(u_, tp[:], st)
            m_ = scr[0:rows, S_MG:S_MG + C]
            nc.gpsimd.tensor_scalar_min(m_, u_, 0.0)
            nc.vector.scalar_tensor_tensor(vv, m_, wF[0:rows, F_GANK + gchunk:F_GANK + gchunk + 1],
                                           u_, OP.mult, OP.add)
        # transpose t -> tT [c, (n,k)]
        for cc in range(CC):
            tt = scr[:, S_TT + cc * KN:S_TT + (cc + 1) * KN]
            pA = cps.tile([128, KN], BF16, tag="tr", bufs=2)
            nc.tensor.transpose(pA[:, 0:128], vA[:, cc * 128:(cc + 1) * 128], idn)
            nc.tensor.transpose(pA[:, 128:KN], vB[:, cc * 128:(cc + 1) * 128], idn[0:24, 0:24])
            nc.scalar.copy(tt[:], pA[:])
        # w2: local2 = t @ w2T (stack layout out)
        l2A = scr[:, S_L2A:S_L2A + C]
        l2B = scr[0:24, S_L2B:S_L2B + C]
        pl2A = cps.tile([128, C], F32, tag="big")
        for cc in range(CC):
            nc.tensor.matmul(pl2A[:], scr[:, S_TT + cc * KN:S_TT + cc * KN + 128],
                             w2T[:, cc * C:(cc + 1) * C], start=(cc == 0), stop=(cc == CC - 1))
        nc.scalar.copy(l2A, pl2A[:])
        pl2B = cps.tile([24, C], F32, tag="smallB")
        for cc in range(CC):
            nc.tensor.matmul(pl2B[:], scr[:, S_TT + cc * KN + 128:S_TT + cc * KN + 152],
                             w2T[:, cc * C:(cc + 1) * C], start=(cc == 0), stop=(cc == CC - 1))
        nc.scalar.copy(l2B, pl2B[:])
        # fuse -> glob [19, 512], then prelu
        gp = cps.tile([K, C], F32, tag="gAB")
        nc.tensor.matmul(gp[:], wB[:, B_FNK0:B_FNK0 + K], l2A, start=True, stop=False)
        nc.tensor.matmul(gp[:], wB[0:24, B_FNK1:B_FNK1 + K], l2B, start=False, stop=True)
        glob = scr[0:K, S_GL:S_GL + C]
        u_ = scr[0:K, S_UG2:S_UG2 + C]
        nc.vector.tensor_scalar_add(u_, gp[:], wF[0:K, F_FB:F_FB + 1])
        m_ = scr[0:K, S_MG2:S_MG2 + C]
        nc.gpsimd.tensor_scalar_min(m_, u_, 0.0)
        nc.vector.scalar_tensor_tensor(glob, m_, wF[0:K, F_RAM1:F_RAM1 + 1], u_, OP.mult, OP.add)
        # globT + val (+ v_b via ones-row matmul)
        valp = cps.tile([K, CI], F32, tag="gAB")
        for cc in range(CC):
            gt = scr[:, S_GT + cc * K:S_GT + (cc + 1) * K]
            pA = cps.tile([128, K], BF16, tag="tr", bufs=2)
            nc.tensor.transpose(pA[:], glob[:, cc * 128:(cc + 1) * 128], idn[0:K, 0:K])
            nc.scalar.copy(gt[:, :], pA[:])
            nc.tensor.matmul(valp[:], gt[:], vwT[:, cc * CI:(cc + 1) * CI],
                             start=(cc == 0), stop=False)
        nc.tensor.matmul(valp[:], wB[0:1, B_ONE119:B_ONE119 + K], wB[0:1, B_VB:B_VB + CI],
                         start=False, stop=True)
        nc.scalar.copy(val, valp[:])
        # local2T + keyT (+ k_b per-partition bias)
        for cc in range(CC):
            lt = scr[:, S_L2T + cc * KN:S_L2T + (cc + 1) * KN]
            pA = cps.tile([128, KN], BF16, tag="tr", bufs=2)
            nc.tensor.transpose(pA[:, 0:128], l2A[:, cc * 128:(cc + 1) * 128], idn)
            nc.tensor.transpose(pA[:, 128:KN], l2B[:, cc * 128:(cc + 1) * 128], idn[0:24, 0:24])
            nc.vector.tensor_copy(lt[:], pA[:])
        for ic in range(IC):
            kp = cps.tile([128, KN], F32, tag="keyp", bufs=1)
            for cc in range(CC):
                nc.tensor.matmul(kp[:], kwT[:, cc * CI + ic * 128: cc * CI + (ic + 1) * 128],
                                 scr[:, S_L2T + cc * KN:S_L2T + (cc + 1) * KN],
                                 start=(cc == 0), stop=(cc == CC - 1))
            nc.scalar.activation(keyT[:, ic * KN:(ic + 1) * KN], kp[:], ACT.Identity,
                                 bias=wF[:, F_KB + ic:F_KB + ic + 1])
    gpool.release()

    # ---------------- phase D: attention + y (single pass) ----------------
    with tc.tile_pool(name="phD_sb", bufs=1) as dsb, \
         tc.tile_pool(name="phD_ps", bufs=1, space="PSUM") as dps:
        for n in range(NB):
            qT = dsb.tile([128, IC * P], BF16, tag="qT", bufs=2)
            for ic in range(IC):
                for nh in range(2):
                    qp = dps.tile([128, 512], F32, tag="qp", bufs=2)
                    for cc in range(CC):
                        xsl = x_sb[cc][:, n * P + nh * 512: n * P + (nh + 1) * 512]
                        nc.tensor.matmul(qp[:], qwT[:, cc * CI + ic * 128: cc * CI + (ic + 1) * 128],
                                         xsl, start=(cc == 0), stop=(cc == CC - 1))
                    qsl = qT[:, ic * P + nh * 512: ic * P + (nh + 1) * 512]
                    if nh == 0:
                        nc.scalar.activation(qsl, qp[:], ACT.Identity,
                                             bias=wF[:, F_QB + ic:F_QB + ic + 1])
                    else:
                        nc.vector.tensor_scalar_add(qsl, qp[:],
                                                    wF[:, F_QB + ic:F_QB + ic + 1])
            ehat = dsb.tile([K, P], BF16, tag="ehat", bufs=2)
            sp = dps.tile([1, P], F32, tag="aop", bufs=1)
            for nh in range(2):
                afp = dps.tile([K, 512], F32, tag="soft", bufs=2)
                for ic in range(IC):
                    ksel = keyT[:, ic * KN + n * K: ic * KN + (n + 1) * K]
                    nc.tensor.matmul(afp[:], ksel, qT[:, ic * P + nh * 512: ic * P + (nh + 1) * 512],
                                     start=(ic == 0), stop=(ic == IC - 1))
                esl = ehat[:, nh * 512:(nh + 1) * 512]
                nc.scalar.activation(esl, afp[:], ACT.Exp)
                nc.tensor.matmul(sp[:, nh * 512:(nh + 1) * 512],
                                 wB[0:K, B_ONE191:B_ONE191 + 1], esl,
                                 start=True, stop=True)
            rrow = dsD[0:1, D_RROW:D_RROW + P]
            nc.vector.reciprocal(rrow, sp[:])
            rrowB = dsb.tile([1, P], BF16, tag="rrowB", bufs=2)
            nc.vector.tensor_copy(rrowB[:], rrow)
            for nh in range(2):
                rbp = dps.tile([K, 512], F32, tag="soft", bufs=2)
                nc.tensor.matmul(rbp[:], wB[0:1, B_ONE119:B_ONE119 + K],
                                 rrowB[:, nh * 512:(nh + 1) * 512],
                                 start=True, stop=True)
                nc.vector.tensor_mul(ehat[:, nh * 512:(nh + 1) * 512],
                                     ehat[:, nh * 512:(nh + 1) * 512], rbp[:])
            at = []
            for ic in range(IC):
                a_ = dsb.tile([128, P], BF16, tag=f"at{ic}", bufs=2)
                aop = dps.tile([128, P], F32, tag="aop", bufs=1)
                for nh in range(2):
                    nc.tensor.matmul(aop[:, nh * 512:(nh + 1) * 512],
                                     val[:, ic * 128:(ic + 1) * 128],
                                     ehat[:, nh * 512:(nh + 1) * 512], start=True, stop=True)
                # attn row-sums ride the copy: mu comes from outw @ sum(attn)
                rcol = dsD[:, D_YS + ic * NB + n: D_YS + ic * NB + n + 1]
                nc.scalar.activation(a_[:], aop[:], ACT.Copy, accum_out=rcol)
                at.append(a_)
            for cc in range(CC):
                for nh in range(2):
                    yp = dps.tile([128, 512], F32, tag="yp", bufs=2)
                    for ic in range(IC):
                        nc.tensor.matmul(yp[:], outwT[:, ic * C + cc * 128: ic * C + (cc + 1) * 128],
                                         at[ic][:, nh * 512:(nh + 1) * 512],
                                         start=(ic == 0), stop=(ic == IC - 1))
                    ysl = y_sb[cc][:, n * P + nh * 512: n * P + (nh + 1) * 512]
                    if nh == 0:
                        nc.vector.tensor_copy(ysl, yp[:])
                    else:
                        nc.scalar.copy(ysl, yp[:])
                # y^2 sums from the bf16-resident y, one act per (cc, bin)
                sq = dsb.tile([128, P], BF16, tag="ysq", bufs=2)
                col2 = D_SQ + cc * NB + n
                nc.scalar.activation(sq[:], y_sb[cc][:, n * P:(n + 1) * P],
                                     ACT.Square, accum_out=dsD[:, col2:col2 + 1])
        # mu: Sigma_y per cc = outwT @ (per-ic attn row sums, summed over bins)
        rs2 = dsD[:, D_YS:D_YS + 2 * NB].rearrange("p (i b) -> p i b", i=IC)
        nc.vector.tensor_reduce(dsD[:, D_YS + 16:D_YS + 16 + IC], rs2, axis=AX.X, op=OP.add)
        rsB = dsb.tile([128, IC], BF16, tag="rsB")
        nc.vector.tensor_copy(rsB[:], dsD[:, D_YS + 16:D_YS + 16 + IC])
        for cc in range(CC):
            mup = dps.tile([128, 1], F32, tag="yp", bufs=2)
            for ic in range(IC):
                nc.tensor.matmul(mup[:], outwT[:, ic * C + cc * 128: ic * C + (cc + 1) * 128],
                                 rsB[:, ic:ic + 1],
                                 start=(ic == 0), stop=(ic == IC - 1))
            nc.vector.tensor_copy(dsD[:, D_ST + 2 * cc:D_ST + 2 * cc + 1], mup[:])
    sqsum = dsD[:, D_SQ:D_SQ + 32].rearrange("p (c b) -> p c b", c=CC)
    st2 = dsD[:, D_ST:D_ST + 2 * CC].rearrange("p (c two) -> p c two", two=2)
    nc.vector.tensor_reduce(st2[:, :, 1], sqsum, axis=AX.X, op=OP.add)

    # ---------------- collective ----------------
    with tc.tile_pool(name="cdram", bufs=1, space="DRAM") as cdram:
        arin = cdram.tile([128, 2 * CC], F32)
        arout = cdram.tile([128, 2 * CC], F32)
        nc.sync.dma_start(arin[:], dsD[:, D_ST:D_ST + 2 * CC])
        nc.gpsimd.collective_compute(
            "AllReduce", OP.add,
            ins=[arin.opt()], outs=[arout.opt()],
            replica_groups=[list(range(n_cores))],
        )
        nc.sync.dma_start(dsD[:, D_SBN:D_SBN + 2 * CC], arout[:])

    # ---------------- BN finalize ----------------
    mom = dsD[:, D_MOM:D_MOM + 2 * CC]
    nc.scalar.mul(mom, dsD[:, D_SBN:D_SBN + 2 * CC], 1.0 / Ntot)
    muv = mom.rearrange("p (c two) -> p c two", two=2)[:, :, 0]
    msq = mom.rearrange("p (c two) -> p c two", two=2)[:, :, 1]
    nc.vector.tensor_mul(dsD[:, D_MUSQ:D_MUSQ + CC], muv, muv)
    nc.vector.tensor_sub(dsD[:, D_VAR:D_VAR + CC], msq, dsD[:, D_MUSQ:D_MUSQ + CC])
    nc.scalar.activation(dsD[:, D_SD:D_SD + CC], dsD[:, D_VAR:D_VAR + CC], ACT.Sqrt,
                         bias=wF[:, F_EPS:F_EPS + 1])
    nc.vector.reciprocal(dsD[:, D_RSTD:D_RSTD + CC], dsD[:, D_SD:D_SD + CC])
    scol = dsD[:, D_SCOL:D_SCOL + CC]
    bcol = dsD[:, D_BCOL:D_BCOL + CC]
    nc.vector.tensor_mul(scol, wF[:, F_GAMMA:F_GAMMA + CC], dsD[:, D_RSTD:D_RSTD + CC])
    nc.vector.tensor_scalar_mul(dsD[:, D_NSC:D_NSC + CC], scol, -1.0)
    for cc in range(CC):
        nc.vector.scalar_tensor_tensor(bcol[:, cc:cc + 1], muv[:, cc:cc + 1],
                                       dsD[:, D_NSC + cc:D_NSC + cc + 1],
                                       wF[:, F_BETA + cc:F_BETA + cc + 1], OP.mult, OP.add)

    # ---------------- phase F: scale/shift + PReLU + residual ----------------
    yv = y_d.rearrange("c h w -> c (h w)")
    stage2 = spool.tile([128, RH * W], BF16, name="stage2")
    with tc.tile_pool(name="phF_sb", bufs=1) as fsb:
        stage1 = spool.tile([128, RH * W], BF16, tag="slot", name="stage1")
        stages = [stage1, stage2]
        for bi in range(BH):
            for cc in range(CC):
                st_ = stages[(bi * CC + cc) % 2]
                stv = st_[:].rearrange("p (h w) -> p h w", w=W)
                for bj in range(BW):
                    n = bi * BW + bj
                    ysl = y_sb[cc][:, n * P:(n + 1) * P]
                    u = fsb.tile([128, P], BF16, tag="u_f", bufs=3)
                    nc.scalar.activation(u[:], ysl, ACT.Identity,
                                         bias=bcol[:, cc:cc + 1], scale=scol[:, cc:cc + 1])
                    m2 = fsb.tile([128, P], BF16, tag="m_f", bufs=3)
                    nc.gpsimd.tensor_scalar_min(m2[:], u[:], 0.0)
                    xpu = fsb.tile([128, P], BF16, tag="xpu_f", bufs=3)
                    nc.vector.scalar_tensor_tensor(xpu[:], m2[:],
                                                   wF[:, F_OAM1 + cc:F_OAM1 + cc + 1],
                                                   u[:], OP.mult, OP.add)
                    dst = stv[:, :, RW * bj:RW * (bj + 1)]
                    xres = x_sb[cc][:, n * P:(n + 1) * P]
                    if bj % 2 == 0:
                        nc.vector.tensor_add(dst, xpu[:], xres)
                    else:
                        nc.gpsimd.tensor_add(dst, xpu[:], xres)
                nc.sync.dma_start(yv[cc * 128:(cc + 1) * 128, RH * bi * W:RH * (bi + 1) * W],
                                  st_[:])
    spool.release()
    xpool.release()
    ypool.release()
    attw.release()
    dpool.release()
    wpool.release()


# ======================================================================
# Entry point: kernel(**inputs) -> np.ndarray [8, 512, 64, 128]
# ======================================================================
import concourse.bacc as bacc
import concourse.tile as tile
from concourse.bass_utils import run_bass_kernel_spmd

N_CORES = 8
_cached = {}


def _build_program(n_cores=N_CORES):
    if "nc" in _cached:
        return _cached["nc"]
    nc = bacc.Bacc("TRN2", target_bir_lowering=False, debug=False, num_devices=n_cores)
    ins = {"x": nc.dram_tensor("x", [C, HWp], BF16, kind="ExternalInput").ap()}
    for nm, shape, dt in WEIGHT_SPECS:
        ins[nm] = nc.dram_tensor(nm, shape, dt, kind="ExternalInput").ap()
    outs = {"y": nc.dram_tensor("y", [C, H, W], BF16, kind="ExternalOutput").ap()}
    with tile.TileContext(nc) as tc:
        build_caam(tc, outs, ins, n_cores)
    nc.compile()
    _cached["nc"] = nc
    return nc


def make_in_maps(inputs):
    x = np.ascontiguousarray(np.asarray(inputs["x"], np.float32))
    prep = host_prep(inputs)
    bf = ml_dtypes.bfloat16
    in_maps = []
    for c in range(N_CORES):
        # bin-blocked: [C, H, W] -> [C, bi, ph, bj, pw] -> [C, (bi bj ph pw)]
        xb = x[c].reshape(C, BH, RH, BW, RW).transpose(0, 1, 3, 2, 4)
        d = {"x": np.ascontiguousarray(xb.reshape(C, HWp)).astype(bf)}
        for nm, _, _ in WEIGHT_SPECS:
            d[nm] = prep[nm]
        in_maps.append(d)
    return in_maps


def kernel(**inputs):
    nc = _build_program()
    in_maps = make_in_maps(inputs)
    res = run_bass_kernel_spmd(nc, in_maps, core_ids=list(range(N_CORES)))
    return np.stack([res.results[c]["y"] for c in range(N_CORES)]).astype(np.float32)


# revision 28
# speedup vs baseline: 15.7654x; 7.1078x over previous
"""CAAM kernel for Trainium2: bf16 single-pass design.

Per-core: one batch element. x arrives host-permuted to bin-blocked
layout [C, 8192] bf16 (free index = bin*1024 + ph*32 + pw). Layouts:
  x resident:  4 SBUF tiles [128, 8192] bf16
  y resident:  4 SBUF tiles [128, 8192] bf16 (attention output, pre-BN)
  camE [19, 8192] bf16: cam, then exp(cam) in place (shares slot with
  phase-C scratch and one phase-F stage buffer)
  ETall [128, 64*19] bf16: exp(cam) transposed per 128-pixel chunk
  stack [(k*8+n), c] bf16 (GCN input), chunk0 rows 0:128, chunk1 0:24
  keyT [128i, 2*152] + val [19, 256] bf16
  per bin: qT [128, 2*1024] bf16, Ehat [19, 1024] bf16 (normalized aff),
  attnT 2x [128, 1024] bf16, y psum -> y resident + sum/sumsq riders.
  BN stats allreduced; phase F: scale/shift + PReLU + residual from
  resident y/x, staged per (bi, cc) for contiguous output DMA.
"""

import numpy as np
import ml_dtypes
import concourse.bass as bass
import concourse.mybir as mybir

F32 = mybir.dt.float32
BF16 = mybir.dt.bfloat16
AX = mybir.AxisListType
OP = mybir.AluOpType
ACT = mybir.ActivationFunctionType

B, C, H, W = 8, 512, 64, 128
K, BH, BW = 19, 2, 4
NB = BH * BW          # 8
CI = C // 2           # 256
HWp = H * W           # 8192
RH, RW = H // BH, W // BW   # 32, 32
P = RH * RW           # 1024
CC = C // 128         # 4
IC = CI // 128        # 2
KN = K * NB           # 152
NPC = HWp // 128      # 64 pixel chunks
EPS = 1e-5

# -------- wpackB column map (bf16 consts) --------
B_IDN = 0        # 128 cols            identity
B_WCAM = 128     # 76 = 4 x 19         conv_cam lhsT chunks
B_W1NK0 = 204    # 152, rows 0:128     gcn conv1 lhsT chunk0
B_W1NK1 = 356    # 152, rows 0:24      chunk1
B_FNK0 = 508     # 19, rows 0:128      fuse lhsT chunk0
B_FNK1 = 527     # 19, rows 0:24       chunk1
B_ONE191 = 546   # 1 col, rows 0:19    ones (sum-over-k lhsT)
B_ONE119 = 547   # 19 cols, row 0      ones (broadcast lhsT)
B_VB = 566       # 256 cols, row 0     v_b
NBW = 822

# -------- wpackF column map (fp32 consts, [128, 22]) --------
F_GANK = 0       # 2: gcn_a-1 per stack row (chunk0, chunk1)
F_CAMB = 2       # 1, rows 0:19
F_FB = 3         # 1, rows 0:19  fuse_b
F_RAM1 = 4       # 1, rows 0:19  relu_a - 1
F_KB = 5         # 2             k_b chunks
F_QB = 7         # 2             q_b chunks
F_GAMMA = 9      # 4
F_BETA = 13      # 4
F_OAM1 = 17      # 4  out_a - 1
F_EPS = 21       # 1
NFW = 22

# -------- dsA ([128, 72]): phase A stats (rows 0:19) --------
A_CS = 0         # 16: cam sums per 512-chunk
A_ES = 16        # 16: exp sums per 512-chunk
A_CSB = 32       # 8: cam sums per bin
A_ESB = 40       # 8: exp sums per bin
A_CLS = 48       # 8: sigmoid(mean cam)
A_REC = 56       # 8: 1/esum
A_SCL = 64       # 8: cls * rec
NA = 72

# -------- dsD ([128, 704]) --------
D_YS = 0         # 64: y sums per (cc, bin, nh)
D_SQ = 64        # 64: y^2 sums per (cc, bin, nh)
D_ST = 128       # 8: packed allreduce input (sum, sumsq per cc)
D_SBN = 136      # 8: allreduce output
D_MOM = 144      # 8
D_VAR = 152      # 4
D_MUSQ = 156     # 4
D_SD = 160       # 4
D_RSTD = 164     # 4
D_SCOL = 168     # 4
D_BCOL = 172     # 4
D_NSC = 176      # 4
D_RROW = 192     # 1024, row 0
ND = 1216

# -------- scr column map (phase-C scratch, bf16, shares camE slot) ----
S_VA = 0         # 512   prelu'd t, chunk0
S_VB = 512       # 512, rows 0:24  chunk1
S_UG = 1024      # 512   u scratch (fp32 would be better but bf16 ok)
S_MG = 1536      # 512
S_TT = 2048      # 608 = 4 x 152   t transposed
S_L2A = 2656     # 512
S_L2B = 3168     # 512, rows 0:24
S_GL = 3680      # 512, rows 0:19  glob (prelu'd)
S_UG2 = 4192     # 512
S_MG2 = 4704     # 512
S_GT = 5216      # 76 = 4 x 19     glob transposed
S_L2T = 5292     # 608             local2 transposed
NS = 5900

# attw pack: keyT 0:304, val 304:560
AT_KEYT = 0
AT_VAL = 304
NAT = 560


def host_prep(wts: dict) -> dict:
    w1 = np.asarray(wts["gcn_w1"], np.float32)
    ga = np.asarray(wts["gcn_a"], np.float32)
    fw = np.asarray(wts["fuse_w"], np.float32).reshape(-1)
    fb = float(np.asarray(wts["fuse_b"], np.float32).reshape(-1)[0])
    ra = float(np.asarray(wts["relu_a"], np.float32).reshape(-1)[0])

    wB = np.zeros((128, NBW), np.float32)
    wB[:, B_IDN:B_IDN + 128] = np.eye(128, dtype=np.float32)
    wcamT = np.asarray(wts["conv_cam_w"], np.float32).T    # [512, 19]
    for cc in range(CC):
        wB[:, B_WCAM + cc*K:B_WCAM + (cc+1)*K] = wcamT[cc*128:(cc+1)*128]
    # conv1 lhsT: W[(m*19+kp), (n*19+k)] = w1[n, m] * (kp == k)
    W1NK = np.zeros((KN, KN), np.float32)
    FNK = np.zeros((KN, K), np.float32)
    ga_nk = np.zeros(KN, np.float32)
    for n in range(NB):
        for k in range(K):
            for m in range(NB):
                W1NK[m*K + k, n*K + k] = w1[n, m]
            FNK[n*K + k, k] = fw[n]
            ga_nk[n*K + k] = ga[n] - 1.0
    wB[:, B_W1NK0:B_W1NK0 + KN] = W1NK[0:128]
    wB[0:24, B_W1NK1:B_W1NK1 + KN] = W1NK[128:KN]
    wB[:, B_FNK0:B_FNK0 + K] = FNK[0:128]
    wB[0:24, B_FNK1:B_FNK1 + K] = FNK[128:KN]
    wB[0:K, B_ONE191] = 1.0
    wB[0, B_ONE119:B_ONE119 + K] = 1.0
    wB[0, B_VB:B_VB + CI] = np.asarray(wts["v_b"], np.float32)

    wF = np.zeros((128, NFW), np.float32)
    wF[:, F_GANK] = ga_nk[0:128]
    wF[0:24, F_GANK + 1] = ga_nk[128:KN]
    wF[0:K, F_CAMB] = np.asarray(wts["conv_cam_b"], np.float32)
    wF[0:K, F_FB] = fb
    wF[0:K, F_RAM1] = ra - 1.0
    wF[:, F_KB:F_KB + 2] = np.asarray(wts["k_b"], np.float32).reshape(IC, 128).T
    wF[:, F_QB:F_QB + 2] = np.asarray(wts["q_b"], np.float32).reshape(IC, 128).T
    wF[:, F_GAMMA:F_GAMMA + 4] = np.asarray(wts["bn_gamma"], np.float32).reshape(CC, 128).T
    wF[:, F_BETA:F_BETA + 4] = np.asarray(wts["bn_beta"], np.float32).reshape(CC, 128).T
    wF[:, F_OAM1:F_OAM1 + 4] = (np.asarray(wts["out_a"], np.float32) - 1.0).reshape(CC, 128).T
    wF[:, F_EPS] = EPS

    bf = ml_dtypes.bfloat16
    return {
        "wpackB": wB.astype(bf), "wpackF": wF,
        "w2T": np.ascontiguousarray(np.asarray(wts["gcn_w2"], np.float32).T).astype(bf),
        "kwT": np.ascontiguousarray(np.asarray(wts["k_w"], np.float32).T).astype(bf),
        "vwT": np.ascontiguousarray(np.asarray(wts["v_w"], np.float32).T).astype(bf),
        "qwT": np.ascontiguousarray(np.asarray(wts["q_w"], np.float32).T).astype(bf),
        "outwT": np.ascontiguousarray(np.asarray(wts["out_w"], np.float32).T).astype(bf),
    }


WEIGHT_SPECS = [
    ("wpackB", [128, NBW], BF16), ("wpackF", [128, NFW], F32),
    ("w2T", [C, C], BF16), ("kwT", [C, CI], BF16), ("vwT", [C, CI], BF16),
    ("qwT", [C, CI], BF16), ("outwT", [CI, C], BF16),
]


def _load_chunked(nc, pool, ap, r, cdim, name, dt=BF16):
    """DRAM [r, cdim] (r = n*128) -> SBUF [128, n*cdim], column-grouped."""
    nchunk = r // 128
    t = pool.tile([128, nchunk * cdim], dt, name=name)
    src = ap.rearrange("(n p) c -> p n c", p=128)
    nc.sync.dma_start(t[:].rearrange("p (n c) -> p n c", n=nchunk), src)
    return t


def build_caam(tc, outs, ins, n_cores):
    nc = tc.nc
    x_d = ins["x"]
    y_d = outs["y"]
    Ntot = float(n_cores * HWp)

    # ---------------- pool stack (LIFO) ----------------
    wpool = tc.alloc_tile_pool(name="wts", bufs=1)
    dpool = tc.alloc_tile_pool(name="stats", bufs=1)
    attw = tc.alloc_tile_pool(name="attw", bufs=1)
    ypool = tc.alloc_tile_pool(name="y_res", bufs=1)
    xpool = tc.alloc_tile_pool(name="x_res", bufs=1)
    spool = tc.alloc_tile_pool(name="slot", bufs=1)
    gpool = tc.alloc_tile_pool(name="gcn", bufs=1)

    wB = wpool.tile([128, NBW], BF16, name="wpackB")
    nc.sync.dma_start(wB[:], ins["wpackB"])
    wF = wpool.tile([128, NFW], F32, name="wpackF")
    nc.sync.dma_start(wF[:], ins["wpackF"])
    qwT = _load_chunked(nc, wpool, ins["qwT"], C, CI, "qwT")
    outwT = _load_chunked(nc, wpool, ins["outwT"], CI, C, "outwT")
    idn = wB[:, B_IDN:B_IDN + 128]

    dsA = dpool.tile([128, NA], F32, name="dsA")
    dsD = dpool.tile([128, ND], F32, name="dsD")

    # x resident, bin-blocked bf16; chunk-interleaved loads so phase A can
    # start after the first wave instead of after the full 8 MB
    x_sb = [xpool.tile([128, HWp], BF16, name=f"x_{cc}") for cc in range(CC)]
    XQ = HWp // 4
    for q in range(4):
        for cc in range(CC):
            nc.sync.dma_start(x_sb[cc][:, q * XQ:(q + 1) * XQ],
                              x_d[cc * 128:(cc + 1) * 128, q * XQ:(q + 1) * XQ])
    y_sb = [ypool.tile([128, HWp], BF16, name=f"y_{cc}") for cc in range(CC)]

    camE = spool.tile([K, HWp], BF16, tag="slot", name="camE")

    # ---------------- phase A: CAM + exp + bin stats ----------------
    with tc.tile_pool(name="phA_ps", bufs=1, space="PSUM") as aps:
        for ch in range(HWp // 512):
            cp = aps.tile([K, 512], F32, tag="camps", bufs=2)
            for cc in range(CC):
                nc.tensor.matmul(cp[:], wB[:, B_WCAM + cc*K:B_WCAM + (cc+1)*K],
                                 x_sb[cc][:, ch * 512:(ch + 1) * 512],
                                 start=(cc == 0), stop=(cc == CC - 1))
            nc.scalar.activation(camE[:, ch * 512:(ch + 1) * 512], cp[:],
                                 ACT.Identity, bias=wF[0:K, F_CAMB:F_CAMB + 1],
                                 accum_out=dsA[0:K, A_CS + ch:A_CS + ch + 1])
            nc.scalar.activation(camE[:, ch * 512:(ch + 1) * 512],
                                 camE[:, ch * 512:(ch + 1) * 512], ACT.Exp,
                                 accum_out=dsA[0:K, A_ES + ch:A_ES + ch + 1])
    cs2 = dsA[0:K, A_CS:A_CS + 16].rearrange("p (n two) -> p n two", two=2)
    nc.vector.tensor_add(dsA[0:K, A_CSB:A_CSB + NB], cs2[:, :, 0], cs2[:, :, 1])
    es2 = dsA[0:K, A_ES:A_ES + 16].rearrange("p (n two) -> p n two", two=2)
    nc.vector.tensor_add(dsA[0:K, A_ESB:A_ESB + NB], es2[:, :, 0], es2[:, :, 1])
    nc.scalar.activation(dsA[0:K, A_CLS:A_CLS + NB], dsA[0:K, A_CSB:A_CSB + NB],
                         ACT.Sigmoid, scale=1.0 / P)
    nc.vector.reciprocal(dsA[0:K, A_REC:A_REC + NB], dsA[0:K, A_ESB:A_ESB + NB])
    nc.vector.tensor_mul(dsA[0:K, A_SCL:A_SCL + NB],
                         dsA[0:K, A_CLS:A_CLS + NB], dsA[0:K, A_REC:A_REC + NB])

    # ---------------- phase B: E^T and per-bin local ----------------
    # ETall[:, pc*19:(pc+1)*19] = camE[:, pc*128:(pc+1)*128]^T
    ETall = gpool.tile([128, NPC * K], BF16, name="ETall")
    stack = gpool.tile([128, 2 * C], BF16, name="stack")
    stackA = stack[:, 0:C]
    stackB = stack[0:24, C:2 * C]
    with tc.tile_pool(name="phB_sb", bufs=1) as bsb, \
         tc.tile_pool(name="phB_ps", bufs=1, space="PSUM") as bps:
        for g in range(NPC // 4):  # 4 transposes per psum tile, 1 copy
            # 20-col slots: bf16 PSUM writes must be 4-byte aligned
            tE = bps.tile([128, 4 * 20], BF16, tag="tE", bufs=2)
            for j in range(4):
                pc = g * 4 + j
                nc.tensor.transpose(tE[:, j * 20:j * 20 + K],
                                    camE[0:K, pc * 128:(pc + 1) * 128],
                                    idn[0:K, 0:K])
            dstE = ETall[:, g * 4 * K:(g + 1) * 4 * K].rearrange("p (j k) -> p j k", j=4)
            srcE = tE[:].rearrange("p (j k) -> p j k", j=4)[:, :, 0:K]
            nc.vector.tensor_copy(dstE, srcE)
        for n in range(NB):
            locp = bps.tile([K, C], F32, tag="locp", bufs=2)
            for pq in range(4):
                xps = bps.tile([128, 2 * C], BF16, tag="xps", bufs=2)
                for half in range(2):
                    p0 = n * P + (pq * 2 + half) * 128
                    for cc in range(CC):
                        nc.tensor.transpose(xps[:, half * C + cc * 128: half * C + (cc + 1) * 128],
                                            x_sb[cc][:, p0:p0 + 128], idn)
                xpp = bsb.tile([128, 2 * C], BF16, tag="xpp", bufs=3)
                if pq % 2 == 0:
                    nc.scalar.copy(xpp[:], xps[:])
                else:
                    nc.vector.tensor_copy(xpp[:], xps[:])
                for half in range(2):
                    pc = pq * 2 + half
                    nc.tensor.matmul(locp[:], ETall[:, (n * 8 + pc) * K:(n * 8 + pc + 1) * K],
                                     xpp[:, half * C:(half + 1) * C],
                                     start=(pc == 0), stop=(pc == 7))
            locS = bsb.tile([K, C], BF16, tag="locS", bufs=2)
            nc.vector.tensor_single_scalar(locS[:], locp[:],
                                           dsA[0:K, A_SCL + n:A_SCL + n + 1], OP.mult)
            # stack rows n*19 .. n*19+19 (may straddle chunks at p=128)
            p0, p1 = n * K, n * K + K
            if p1 <= 128:
                nc.sync.dma_start(stackA[p0:p1, :], locS[:, :])
            elif p0 >= 128:
                nc.sync.dma_start(stackB[p0 - 128:p1 - 128, :], locS[:, :])
            else:
                nc.sync.dma_start(stackA[p0:128, :], locS[0:128 - p0, :])
                nc.sync.dma_start(stackB[0:p1 - 128, :], locS[128 - p0:K, :])

    # ---------------- phase C: GCN + fuse + key/val ----------------
    atp = attw.tile([128, NAT], BF16, name="attpack")
    keyT = atp[:, AT_KEYT:AT_KEYT + IC * KN]
    val = atp[0:K, AT_VAL:AT_VAL + CI]
    scr = spool.tile([128, NS], BF16, tag="slot", name="scr")
    vA = scr[:, S_VA:S_VA + C]
    vB = scr[0:24, S_VB:S_VB + C]
    with tc.tile_pool(name="phC_sb", bufs=1) as csb, \
         tc.tile_pool(name="phC_ps", bufs=1, space="PSUM") as cps:
        w2T = _load_chunked(nc, csb, ins["w2T"], C, C, "w2T")
        kwT = _load_chunked(nc, csb, ins["kwT"], C, CI, "kwT")
        vwT = _load_chunked(nc, csb, ins["vwT"], C, CI, "vwT")
        # conv1: t = W1NK.T @ stack (contraction over 152 stack rows)
        tpA = cps.tile([128, C], F32, tag="big")
        nc.tensor.matmul(tpA[:], wB[:, B_W1NK0:B_W1NK0 + 128], stackA, start=True, stop=False)
        nc.tensor.matmul(tpA[:], wB[0:24, B_W1NK1:B_W1NK1 + 128], stackB, start=False, stop=True)
        tpB = cps.tile([24, C], F32, tag="smallB")
        nc.tensor.matmul(tpB[:], wB[:, B_W1NK0 + 128:B_W1NK0 + KN], stackA, start=True, stop=False)
        nc.tensor.matmul(tpB[:], wB[0:24, B_W1NK1 + 128:B_W1NK1 + KN], stackB, start=False, stop=True)
        # prelu(t + stack), per-row alpha gcn_a[n] (F_GANK cols)
        for (tp, st, vv, gchunk, rows) in ((tpA, stackA, vA, 0, 128),
                                           (tpB, stackB, vB, 1, 24)):
            u_ = scr[0:rows, S_UG:S_UG + C]
            nc.vector.tensor_add(u_, tp[:], st)
            m_ = scr[0:rows, S_MG:S_MG + C]
            nc.gpsimd.tensor_scalar_min(m_, u_, 0.0)
            nc.vector.scalar_tensor_tensor(vv, m_, wF[0:rows, F_GANK + gchunk:F_GANK + gchunk + 1],
                                           u_, OP.mult, OP.add)
        # transpose t -> tT [c, (n,k)]
        for cc in range(CC):
            tt = scr[:, S_TT + cc * KN:S_TT + (cc + 1) * KN]
            pA = cps.tile([128, KN], BF16, tag="tr", bufs=2)
            nc.tensor.transpose(pA[:, 0:128], vA[:, cc * 128:(cc + 1) * 128], idn)
            nc.tensor.transpose(pA[:, 128:KN], vB[:, cc * 128:(cc + 1) * 128], idn[0:24, 0:24])
            nc.scalar.copy(tt[:], pA[:])
        # w2: local2 = t @ w2T (stack layout out)
        l2A = scr[:, S_L2A:S_L2A + C]
        l2B = scr[0:24, S_L2B:S_L2B + C]
        pl2A = cps.tile([128, C], F32, tag="big")
        for cc in range(CC):
            nc.tensor.matmul(pl2A[:], scr[:, S_TT + cc * KN:S_TT + cc * KN + 128],
                             w2T[:, cc * C:(cc + 1) * C], start=(cc == 0), stop=(cc == CC - 1))
        nc.scalar.copy(l2A, pl2A[:])
        pl2B = cps.tile([24, C], F32, tag="smallB")
        for cc in range(CC):
            nc.tensor.matmul(pl2B[:], scr[:, S_TT + cc * KN + 128:S_TT + cc * KN + 152],
                             w2T[:, cc * C:(cc + 1) * C], start=(cc == 0), stop=(cc == CC - 1))
        nc.scalar.copy(l2B, pl2B[:])
        # fuse -> glob [19, 512], then prelu
        gp = cps.tile([K, C], F32, tag="gAB")
        nc.tensor.matmul(gp[:], wB[:, B_FNK0:B_FNK0 + K], l2A, start=True, stop=False)
        nc.tensor.matmul(gp[:], wB[0:24, B_FNK1:B_FNK1 + K], l2B, start=False, stop=True)
        glob = scr[0:K, S_GL:S_GL + C]
        u_ = scr[0:K, S_UG2:S_UG2 + C]
        nc.vector.tensor_scalar_add(u_, gp[:], wF[0:K, F_FB:F_FB + 1])
        m_ = scr[0:K, S_MG2:S_MG2 + C]
        nc.gpsimd.tensor_scalar_min(m_, u_, 0.0)
        nc.vector.scalar_tensor_tensor(glob, m_, wF[0:K, F_RAM1:F_RAM1 + 1], u_, OP.mult, OP.add)
        # globT + val (+ v_b via ones-row matmul)
        valp = cps.tile([K, CI], F32, tag="gAB")
        for cc in range(CC):
            gt = scr[:, S_GT + cc * K:S_GT + (cc + 1) * K]
            pA = cps.tile([128, K], BF16, tag="tr", bufs=2)
            nc.tensor.transpose(pA[:], glob[:, cc * 128:(cc + 1) * 128], idn[0:K, 0:K])
            nc.scalar.copy(gt[:, :], pA[:])
            nc.tensor.matmul(valp[:], gt[:], vwT[:, cc * CI:(cc + 1) * CI],
                             start=(cc == 0), stop=False)
        nc.tensor.matmul(valp[:], wB[0:1, B_ONE119:B_ONE119 + K], wB[0:1, B_VB:B_VB + CI],
                         start=False, stop=True)
        nc.scalar.copy(val, valp[:])
        # local2T + keyT (+ k_b per-partition bias)
        for cc in range(CC):
            lt = scr[:, S_L2T + cc * KN:S_L2T + (cc + 1) * KN]
            pA = cps.tile([128, KN], BF16, tag="tr", bufs=2)
            nc.tensor.transpose(pA[:, 0:128], l2A[:, cc * 128:(cc + 1) * 128], idn)
            nc.tensor.transpose(pA[:, 128:KN], l2B[:, cc * 128:(cc + 1) * 128], idn[0:24, 0:24])
            nc.vector.tensor_copy(lt[:], pA[:])
        for ic in range(IC):
            kp = cps.tile([128, KN], F32, tag="keyp", bufs=1)
            for cc in range(CC):
                nc.tensor.matmul(kp[:], kwT[:, cc * CI + ic * 128: cc * CI + (ic + 1) * 128],
                                 scr[:, S_L2T + cc * KN:S_L2T + (cc + 1) * KN],
                                 start=(cc == 0), stop=(cc == CC - 1))
            nc.scalar.activation(keyT[:, ic * KN:(ic + 1) * KN], kp[:], ACT.Identity,
                                 bias=wF[:, F_KB + ic:F_KB + ic + 1])
    gpool.release()

    # ---------------- phase D: attention + y (single pass) ----------------
    with tc.tile_pool(name="phD_sb", bufs=1) as dsb, \
         tc.tile_pool(name="phD_ps", bufs=1, space="PSUM") as dps:
        for n in range(NB):
            qT = dsb.tile([128, IC * P], BF16, tag="qT", bufs=2)
            for ic in range(IC):
                for nh in range(2):
                    qp = dps.tile([128, 512], F32, tag="qp", bufs=2)
                    for cc in range(CC):
                        xsl = x_sb[cc][:, n * P + nh * 512: n * P + (nh + 1) * 512]
                        nc.tensor.matmul(qp[:], qwT[:, cc * CI + ic * 128: cc * CI + (ic + 1) * 128],
                                         xsl, start=(cc == 0), stop=(cc == CC - 1))
                    qsl = qT[:, ic * P + nh * 512: ic * P + (nh + 1) * 512]
                    if nh == 0:
                        nc.scalar.activation(qsl, qp[:], ACT.Identity,
                                             bias=wF[:, F_QB + ic:F_QB + ic + 1])
                    else:
                        nc.vector.tensor_scalar_add(qsl, qp[:],
                                                    wF[:, F_QB + ic:F_QB + ic + 1])
            ehat = dsb.tile([K, P], BF16, tag="ehat", bufs=2)
            sp = dps.tile([1, P], F32, tag="aop", bufs=1)
            for nh in range(2):
                afp = dps.tile([K, 512], F32, tag="soft", bufs=2)
                for ic in range(IC):
                    ksel = keyT[:, ic * KN + n * K: ic * KN + (n + 1) * K]
                    nc.tensor.matmul(afp[:], ksel, qT[:, ic * P + nh * 512: ic * P + (nh + 1) * 512],
                                     start=(ic == 0), stop=(ic == IC - 1))
                esl = ehat[:, nh * 512:(nh + 1) * 512]
                nc.scalar.activation(esl, afp[:], ACT.Exp)
                nc.tensor.matmul(sp[:, nh * 512:(nh + 1) * 512],
                                 wB[0:K, B_ONE191:B_ONE191 + 1], esl,
                                 start=True, stop=True)
            rrow = dsD[0:1, D_RROW:D_RROW + P]
            nc.vector.reciprocal(rrow, sp[:])
            rrowB = dsb.tile([1, P], BF16, tag="rrowB", bufs=2)
            nc.vector.tensor_copy(rrowB[:], rrow)
            for nh in range(2):
                rbp = dps.tile([K, 512], F32, tag="soft", bufs=2)
                nc.tensor.matmul(rbp[:], wB[0:1, B_ONE119:B_ONE119 + K],
                                 rrowB[:, nh * 512:(nh + 1) * 512],
                                 start=True, stop=True)
                nc.vector.tensor_mul(ehat[:, nh * 512:(nh + 1) * 512],
                                     ehat[:, nh * 512:(nh + 1) * 512], rbp[:])
            at = []
            for ic in range(IC):
                a_ = dsb.tile([128, P], BF16, tag=f"at{ic}", bufs=2)
                aop = dps.tile([128, P], F32, tag="aop", bufs=1)
                for nh in range(2):
                    nc.tensor.matmul(aop[:, nh * 512:(nh + 1) * 512],
                                     val[:, ic * 128:(ic + 1) * 128],
                                     ehat[:, nh * 512:(nh + 1) * 512], start=True, stop=True)
                # attn row-sums ride the copy: mu comes from outw @ sum(attn)
                rcol = dsD[:, D_YS + ic * NB + n: D_YS + ic * NB + n + 1]
                nc.scalar.activation(a_[:], aop[:], ACT.Copy, accum_out=rcol)
                at.append(a_)
            for cc in range(CC):
                for nh in range(2):
                    yp = dps.tile([128, 512], F32, tag="yp", bufs=2)
                    for ic in range(IC):
                        nc.tensor.matmul(yp[:], outwT[:, ic * C + cc * 128: ic * C + (cc + 1) * 128],
                                         at[ic][:, nh * 512:(nh + 1) * 512],
                                         start=(ic == 0), stop=(ic == IC - 1))
                    ysl = y_sb[cc][:, n * P + nh * 512: n * P + (nh + 1) * 512]
                    if nh == 0:
                        nc.vector.tensor_copy(ysl, yp[:])
                    else:
                        nc.scalar.copy(ysl, yp[:])
                # y^2 sums from the bf16-resident y, one act per (cc, bin)
                sq = dsb.tile([128, P], BF16, tag="ysq", bufs=2)
                col2 = D_SQ + cc * NB + n
                nc.scalar.activation(sq[:], y_sb[cc][:, n * P:(n + 1) * P],
                                     ACT.Square, accum_out=dsD[:, col2:col2 + 1])
        # mu: Sigma_y per cc = outwT @ (per-ic attn row sums, summed over bins)
        rs2 = dsD[:, D_YS:D_YS + 2 * NB].rearrange("p (i b) -> p i b", i=IC)
        nc.vector.tensor_reduce(dsD[:, D_YS + 16:D_YS + 16 + IC], rs2, axis=AX.X, op=OP.add)
        rsB = dsb.tile([128, IC], BF16, tag="rsB")
        nc.vector.tensor_copy(rsB[:], dsD[:, D_YS + 16:D_YS + 16 + IC])
        for cc in range(CC):
            mup = dps.tile([128, 1], F32, tag="yp", bufs=2)
            for ic in range(IC):
                nc.tensor.matmul(mup[:], outwT[:, ic * C + cc * 128: ic * C + (cc + 1) * 128],
                                 rsB[:, ic:ic + 1],
                                 start=(ic == 0), stop=(ic == IC - 1))
            nc.vector.tensor_copy(dsD[:, D_ST + 2 * cc:D_ST + 2 * cc + 1], mup[:])
    sqsum = dsD[:, D_SQ:D_SQ + 32].rearrange("p (c b) -> p c b", c=CC)
    st2 = dsD[:, D_ST:D_ST + 2 * CC].rearrange("p (c two) -> p c two", two=2)
    nc.vector.tensor_reduce(st2[:, :, 1], sqsum, axis=AX.X, op=OP.add)

    # ---------------- collective ----------------
    with tc.tile_pool(name="cdram", bufs=1, space="DRAM") as cdram:
        arin = cdram.tile([128, 2 * CC], F32)
        arout = cdram.tile([128, 2 * CC], F32)
        nc.sync.dma_start(arin[:], dsD[:, D_ST:D_ST + 2 * CC])
        nc.gpsimd.collective_compute(
            "AllReduce", OP.add,
            ins=[arin.opt()], outs=[arout.opt()],
            replica_groups=[list(range(n_cores))],
        )
        nc.sync.dma_start(dsD[:, D_SBN:D_SBN + 2 * CC], arout[:])

    # ---------------- BN finalize ----------------
    mom = dsD[:, D_MOM:D_MOM + 2 * CC]
    nc.scalar.mul(mom, dsD[:, D_SBN:D_SBN + 2 * CC], 1.0 / Ntot)
    muv = mom.rearrange("p (c two) -> p c two", two=2)[:, :, 0]
    msq = mom.rearrange("p (c two) -> p c two", two=2)[:, :, 1]
    nc.vector.tensor_mul(dsD[:, D_MUSQ:D_MUSQ + CC], muv, muv)
    nc.vector.tensor_sub(dsD[:, D_VAR:D_VAR + CC], msq, dsD[:, D_MUSQ:D_MUSQ + CC])
    nc.scalar.activation(dsD[:, D_SD:D_SD + CC], dsD[:, D_VAR:D_VAR + CC], ACT.Sqrt,
                         bias=wF[:, F_EPS:F_EPS + 1])
    nc.vector.reciprocal(dsD[:, D_RSTD:D_RSTD + CC], dsD[:, D_SD:D_SD + CC])
    scol = dsD[:, D_SCOL:D_SCOL + CC]
    bcol = dsD[:, D_BCOL:D_BCOL + CC]
    nc.vector.tensor_mul(scol, wF[:, F_GAMMA:F_GAMMA + CC], dsD[:, D_RSTD:D_RSTD + CC])
    nc.vector.tensor_scalar_mul(dsD[:, D_NSC:D_NSC + CC], scol, -1.0)
    for cc in range(CC):
        nc.vector.scalar_tensor_tensor(bcol[:, cc:cc + 1], muv[:, cc:cc + 1],
                                       dsD[:, D_NSC + cc:D_NSC + cc + 1],
                                       wF[:, F_BETA + cc:F_BETA + cc + 1], OP.mult, OP.add)

    # ---------------- phase F: scale/shift + PReLU + residual ----------------
    yv = y_d.rearrange("c h w -> c (h w)")
    stage2 = spool.tile([128, RH * W], BF16, name="stage2")
    with tc.tile_pool(name="phF_sb", bufs=1) as fsb:
        stage1 = spool.tile([128, RH * W], BF16, tag="slot", name="stage1")
        stages = [stage1, stage2]
        for bi in range(BH):
            for cc in range(CC):
                st_ = stages[(bi * CC + cc) % 2]
                stv = st_[:].rearrange("p (h w) -> p h w", w=W)
                for bj in range(BW):
                    n = bi * BW + bj
                    ysl = y_sb[cc][:, n * P:(n + 1) * P]
                    u = fsb.tile([128, P], BF16, tag="u_f", bufs=3)
                    nc.scalar.activation(u[:], ysl, ACT.Identity,
                                         bias=bcol[:, cc:cc + 1], scale=scol[:, cc:cc + 1])
                    m2 = fsb.tile([128, P], BF16, tag="m_f", bufs=3)
                    nc.gpsimd.tensor_scalar_min(m2[:], u[:], 0.0)
                    xpu = fsb.tile([128, P], BF16, tag="xpu_f", bufs=3)
                    nc.vector.scalar_tensor_tensor(xpu[:], m2[:],
                                                   wF[:, F_OAM1 + cc:F_OAM1 + cc + 1],
                                                   u[:], OP.mult, OP.add)
                    dst = stv[:, :, RW * bj:RW * (bj + 1)]
                    xres = x_sb[cc][:, n * P:(n + 1) * P]
                    if bj % 2 == 0:
                        nc.vector.tensor_add(dst, xpu[:], xres)
                    else:
                        nc.gpsimd.tensor_add(dst, xpu[:], xres)
                nc.sync.dma_start(yv[cc * 128:(cc + 1) * 128, RH * bi * W:RH * (bi + 1) * W],
                                  st_[:])
    spool.release()
    xpool.release()
    ypool.release()
    attw.release()
    dpool.release()
    wpool.release()


# ======================================================================
# Entry point: kernel(**inputs) -> np.ndarray [8, 512, 64, 128]
# ======================================================================
import concourse.bacc as bacc
import concourse.tile as tile
from concourse.bass_utils import run_bass_kernel_spmd

N_CORES = 8
_cached = {}


def _build_program(n_cores=N_CORES):
    if "nc" in _cached:
        return _cached["nc"]
    nc = bacc.Bacc("TRN2", target_bir_lowering=False, debug=False, num_devices=n_cores)
    ins = {"x": nc.dram_tensor("x", [C, HWp], BF16, kind="ExternalInput").ap()}
    for nm, shape, dt in WEIGHT_SPECS:
        ins[nm] = nc.dram_tensor(nm, shape, dt, kind="ExternalInput").ap()
    outs = {"y": nc.dram_tensor("y", [C, H, W], BF16, kind="ExternalOutput").ap()}
    with tile.TileContext(nc) as tc:
        build_caam(tc, outs, ins, n_cores)
    nc.compile()
    _cached["nc"] = nc
    return nc


def make_in_maps(inputs):
    x = np.ascontiguousarray(np.asarray(inputs["x"], np.float32))
    prep = host_prep(inputs)
    bf = ml_dtypes.bfloat16
    in_maps = []
    for c in range(N_CORES):
        # bin-blocked: [C, H, W] -> [C, bi, ph, bj, pw] -> [C, (bi bj ph pw)]
        xb = x[c].reshape(C, BH, RH, BW, RW).transpose(0, 1, 3, 2, 4)
        d = {"x": np.ascontiguousarray(xb.reshape(C, HWp)).astype(bf)}
        for nm, _, _ in WEIGHT_SPECS:
            d[nm] = prep[nm]
        in_maps.append(d)
    return in_maps


def kernel(**inputs):
    nc = _build_program()
    in_maps = make_in_maps(inputs)
    res = run_bass_kernel_spmd(nc, in_maps, core_ids=list(range(N_CORES)))
    return np.stack([res.results[c]["y"] for c in range(N_CORES)]).astype(np.float32)
